# revision 1
# baseline (speedup 1.0000x reference)
"""DynamicGraphCNN (DGCNN) forward pass on 8 Trainium2 NeuronCores.

Data-parallel over batch B=8: one point cloud per core. Per layer (edge-conv):
  scores  S'[i,j] = <x_i, x_j> - ||x_j||^2/2    (rank-equivalent to -dist^2)
  top-20 neighbors per row via DVE max/max_index/match_replace
  h[i,j] = u_i + v_{n(i,j)} with u = x(Wc-Wn)^T + b, v = x Wn^T
  BN (training stats over B,N,k) from global sums:
      Sum h   = 20*Sum u + Sum_{ij} v_n
      Sum h^2 = 20*Sum u^2 + 2*Sum_i u_i.s_i + Sum_{ij} v_n^2
  computed with bf16 PE matmuls over the gathered tiles (j-packed into psum),
  cross-term via u-weighted matmuls + diagonal-mask extraction.
  Cross-core reduction: one 8-core AllReduce per layer.
  y_i = relu(scale*(u_i + max_j v_n) + shift)   (monotone: max before affine)
Final: global max over points, then linear head.
"""
import sys
sys.path.insert(0, '/opt/trn_rl_repo')

import numpy as np

B, N, K = 8, 2048, 20
NT = N // 128                      # 16 point tiles of 128
LAYERS = [(3, 64), (64, 128), (128, 256)]
NCORES = 8
GATHER_SPLITS = [(0, 1024), (1024, 1024), (2048, 512)]

_BUILT = {}


def _build(stage=6, dbg=False):
    import contextlib
    import concourse.bacc as bacc
    import concourse.mybir as mybir
    import concourse.tile as tile

    f32 = mybir.dt.float32
    f32r = mybir.dt.float32r
    bf16 = mybir.dt.bfloat16
    i16 = mybir.dt.int16
    u32 = mybir.dt.uint32
    AOT = mybir.AluOpType
    AF = mybir.ActivationFunctionType

    nc = bacc.Bacc("TRN2", target_bir_lowering=False, debug=False,
                   num_devices=NCORES)

    # ---------------- external tensors ----------------
    xT_in = nc.dram_tensor("xT", [3, N], f32, kind="ExternalInput")
    ext = {}
    for li, (ci, co) in enumerate(LAYERS):
        ext[f"wcm{li}"] = nc.dram_tensor(f"wcm{li}", [ci, co], f32, kind="ExternalInput")
        ext[f"wn{li}"] = nc.dram_tensor(f"wn{li}", [ci, co], f32, kind="ExternalInput")
        for rn in ("brow", "grow", "berow"):
            ext[f"{rn}{li}"] = nc.dram_tensor(f"{rn}{li}", [1, co], f32, kind="ExternalInput")
        for h in range(-(-co // 128)):
            hc = min(128, co - 128 * h)
            ext[f"mask{li}_{h}"] = nc.dram_tensor(
                f"mask{li}_{h}", [hc, 512], f32, kind="ExternalInput")
    ident_in = nc.dram_tensor("ident", [128, 128], f32, kind="ExternalInput")
    onesr_in = nc.dram_tensor("onesr", [1, 128], f32, kind="ExternalInput")
    woT_in = nc.dram_tensor("woT", [256, 256], f32, kind="ExternalInput")
    bo_in = nc.dram_tensor("boRow", [1, 256], f32, kind="ExternalInput")
    out_ext = nc.dram_tensor("out", [1, 256], f32, kind="ExternalOutput")
    if dbg:
        dbg_maxv = nc.dram_tensor("dbg_maxv", [128, 24], f32, kind="ExternalOutput")
        dbg_idxu = nc.dram_tensor("dbg_idxu", [128, 24], u32, kind="ExternalOutput")
        dbg_dst = nc.dram_tensor("dbg_dst", [128, 20, 64], f32, kind="ExternalOutput")
        dbg_m = nc.dram_tensor("dbg_m", [128, 1024], f32, kind="ExternalOutput")
        dbg_sr = nc.dram_tensor("dbg_sr", [1, 512], f32, kind="ExternalOutput")
        dbg_sg = nc.dram_tensor("dbg_sg", [1, 512], f32, kind="ExternalOutput")
        dbg_y = nc.dram_tensor("dbg_y", [64, 2048], f32, kind="ExternalOutput")

    with tile.TileContext(nc) as tc:
        ctx = contextlib.ExitStack()
        with ctx:
            big = ctx.enter_context(tc.tile_pool(name="big", bufs=3))      # S / ysq
            ytp = ctx.enter_context(tc.tile_pool(name="ytp", bufs=1))      # yT (2 tags)
            allp = ctx.enter_context(tc.tile_pool(name="allp", bufs=1))    # layer residents
            resid = ctx.enter_context(tc.tile_pool(name="resid", bufs=1))  # constants
            dstp = ctx.enter_context(tc.tile_pool(name="dstp", bufs=2))
            bfp = ctx.enter_context(tc.tile_pool(name="bfp", bufs=3))      # dbf / dsq
            small = ctx.enter_context(tc.tile_pool(name="small", bufs=2))  # idx plumbing
            rows = ctx.enter_context(tc.tile_pool(name="rows", bufs=1))    # [1,*] rows
            vcp = ctx.enter_context(tc.tile_pool(name="vcp", bufs=2))      # staging
            dram = ctx.enter_context(tc.tile_pool(name="dram", bufs=1, space="DRAM"))
            pscore = ctx.enter_context(tc.tile_pool(name="pscore", bufs=2, space="PSUM"))
            pyp = ctx.enter_context(tc.tile_pool(name="pyp", bufs=1, space="PSUM"))
            pmix = ctx.enter_context(tc.tile_pool(name="pmix", bufs=2, space="PSUM"))
            pstat = ctx.enter_context(tc.tile_pool(name="pstat", bufs=1, space="PSUM"))

            # ---------- kernel-lifetime constants ----------
            ident = resid.tile([128, 128], f32, tag="ident")
            nc.sync.dma_start(ident[:], ident_in[:])
            onesRow = resid.tile([1, 128], f32, tag="onesRow")
            nc.vector.memset(onesRow[:], 1.0)
            onesColF = resid.tile([128, 1], f32, tag="onesColF")
            nc.vector.memset(onesColF[:], 1.0)
            onesCol_bf = resid.tile([128, 1], bf16, tag="onesColbf")
            nc.vector.memset(onesCol_bf[:], 1.0)
            negHalfCol = resid.tile([128, 1], f32, tag="negHalfCol")
            nc.vector.memset(negHalfCol[:], -0.5)
            woT_sb = resid.tile([128, 2, 256], f32, tag="woT")
            for h in range(2):
                nc.sync.dma_start(woT_sb[:, h, :], woT_in[128 * h:128 * (h + 1), :])
            boRow = resid.tile([1, 256], f32, tag="boRow")
            nc.sync.dma_start(boRow[:], bo_in[:])
            gmax = resid.tile([128, 2, 128], f32, tag="gmax")
            nc.vector.memset(gmax[:], -1e30)

            yT = ytp.tile([128, N], f32, tag="yt0")
            nc.sync.dma_start(yT[0:3, :], xT_in[:])

            nlayers = len(LAYERS) if stage >= 6 else 1
            for li, (CI, CO) in enumerate(LAYERS[:nlayers]):
                NH = -(-CO // 128)
                CH = min(128, CO)
                G = 512 // CO
                jgroups = []
                j0 = 0
                while j0 < K:
                    jgroups.append((j0, min(G, K - j0)))
                    j0 += G
                last_layer = (li == len(LAYERS) - 1)

                # ---------- weights / rows ----------
                wcm = allp.tile([CI, CO], f32, tag="wcm")
                nc.sync.dma_start(wcm[:], ext[f"wcm{li}"][:])
                wn = allp.tile([CI, CO], f32, tag="wn")
                nc.sync.dma_start(wn[:], ext[f"wn{li}"][:])
                brow = allp.tile([1, CO], f32, tag="brow")
                nc.sync.dma_start(brow[:], ext[f"brow{li}"][:])
                grow = allp.tile([1, CO], f32, tag="grow")
                nc.sync.dma_start(grow[:], ext[f"grow{li}"][:])
                berow = allp.tile([1, CO], f32, tag="berow")
                nc.sync.dma_start(berow[:], ext[f"berow{li}"][:])
                masks = []
                for h in range(NH):
                    mk = allp.tile([CH, 512], f32, tag=f"mask{h}")
                    nc.sync.dma_start(mk[:], ext[f"mask{li}_{h}"][:])
                    masks.append(mk)

                # ---------- prep: negxx row via PE ----------
                ysq = big.tile([128, N], f32, tag="big")
                nc.scalar.activation(ysq[0:CI, :], yT[0:CI, :], AF.Square)
                negxx = allp.tile([1, 2048], f32, tag="negxx")
                for nj in range(4):
                    ps = pmix.tile([1, 512], f32, tag="pmix")
                    nc.tensor.matmul(ps[:], negHalfCol[0:CI, :],
                                     ysq[0:CI, nj * 512:(nj + 1) * 512],
                                     start=True, stop=True)
                    nc.scalar.activation(negxx[:, nj * 512:(nj + 1) * 512],
                                         ps[:], AF.Copy)

                # ---------- prep: u, v per tile; v -> vtab ----------
                vtab = dram.tile([N, CO], f32, tag=f"vtab{li}")
                u_all = allp.tile([128, NT, CO], f32, tag="u_all")
                ubf_all = allp.tile([128, NT, CO], bf16, tag="ubf_all")
                for t in range(NT):
                    tsl = slice(t * 128, (t + 1) * 128)
                    psU = pmix.tile([128, CO], f32, tag="pmix")
                    nc.tensor.matmul(psU[:], yT[0:CI, tsl], wcm[:], start=True, stop=False)
                    nc.tensor.matmul(psU[:], onesRow[:, 0:128], brow[:],
                                     start=False, stop=True)
                    nc.scalar.activation(u_all[:, t, :], psU[:], AF.Copy)
                    nc.scalar.activation(ubf_all[:, t, :], psU[:], AF.Copy)
                    psV = pmix.tile([128, CO], f32, tag="pmix")
                    nc.tensor.matmul(psV[:], yT[0:CI, tsl], wn[:],
                                     start=True, stop=True)
                    vst = vcp.tile([128, CO], f32, tag="vst")
                    nc.scalar.activation(vst[:], psV[:], AF.Copy)
                    nc.sync.dma_start(vtab[tsl, :], vst[:])

                # ---------- Sum u / Sum u^2 at prep (fp32, exact) ----------
                psSQu = pstat.tile([33, 512], f32, tag="psSQ", name="psSQu")
                psSu = psSQu[0:1, :]
                psQu = psSQu[32:33, :]
                u_flat = u_all[:].rearrange("p t c -> p (t c)")
                nuv = NT * CO // 512
                for s in range(nuv):
                    usqf = vcp.tile([128, 512], f32, tag="usq")
                    nc.scalar.activation(usqf[:], u_flat[:, 512 * s:512 * (s + 1)],
                                         AF.Square)
                    nc.tensor.matmul(psSu, onesColF[:],
                                     u_flat[:, 512 * s:512 * (s + 1)],
                                     start=(s == 0), stop=(s == nuv - 1),
                                     skip_group_check=True)
                    nc.tensor.matmul(psQu, onesColF[:], usqf[:],
                                     start=(s == 0), stop=(s == nuv - 1),
                                     skip_group_check=True)
                rowSu = rows.tile([1, 512], f32, tag="rowSu")
                nc.scalar.activation(rowSu[:], psSu, AF.Copy)
                rowQu = rows.tile([1, 512], f32, tag="rowQu")
                nc.scalar.activation(rowQu[:], psQu, AF.Copy)

                # ---------- stat psums (locked for the layer) ----------
                psSQ = pstat.tile([33, 512], f32, tag="psSQ", name="psSQm")
                psS = psSQ[0:1, :]
                psQ = psSQ[32:33, :]
                psX = [pstat.tile([CH, 512], f32, tag=f"psX{h}", name=f"psX{li}_{h}") for h in range(NH)]

                yPre = None
                if not last_layer:
                    yPre = big.tile([CH, N], f32, tag="big", name=f"yPre{li}")

                # ---------- main loop: 1-tile software pipeline ----------
                def front(t):
                    tsl = slice(t * 128, (t + 1) * 128)
                    S = big.tile([128, N], f32, tag="big", name=f"S{li}_{t}")
                    for nj in range(4):
                        psSc = pscore.tile([128, 512], f32, tag="psc")
                        nc.tensor.matmul(psSc[:], yT[0:CI, tsl],
                                         yT[0:CI, nj * 512:(nj + 1) * 512],
                                         start=True, stop=False)
                        nc.tensor.matmul(psSc[:], onesRow[:, 0:128],
                                         negxx[:, nj * 512:(nj + 1) * 512],
                                         start=False, stop=True)
                        nc.scalar.activation(S[:, nj * 512:(nj + 1) * 512],
                                             psSc[:], AF.Copy)
                    # top-20: 3 rounds of 8
                    maxv = small.tile([128, 24], f32, tag="maxv")
                    idxu = small.tile([128, 24], u32, tag="idxu")
                    for r in range(3):
                        rs = slice(8 * r, 8 * (r + 1))
                        nc.vector.max(maxv[:, rs], S[:])
                        nc.vector.max_index(idxu[:, rs], maxv[:, rs], S[:])
                        if r < 2:
                            nc.vector.match_replace(S[:], maxv[:, rs], S[:], -1e30)
                    if dbg and li == 0 and t == 0:
                        nc.sync.dma_start(dbg_maxv[:], maxv[:])
                        nc.sync.dma_start(dbg_idxu[:], idxu[:])
                    if stage < 2:
                        return None
                    # index plumbing: [128,20] u32 -> wrapped [128,160] i16
                    idxf = small.tile([128, 20], f32, tag="idxf")
                    nc.vector.tensor_copy(idxf[:], idxu[:, 0:20])
                    psT1 = pmix.tile([20, 128], f32, tag="pmix")
                    nc.tensor.transpose(psT1[:], idxf[:], ident[:])
                    idxT = small.tile([20, 128], f32, tag="idxT")
                    nc.scalar.activation(idxT[:], psT1[:], AF.Copy)
                    psT2 = pmix.tile([16, 8, 20], f32, tag="pmix")
                    for pg in range(8):
                        nc.tensor.transpose(psT2[:, pg, :],
                                            idxT[:, pg * 16:(pg + 1) * 16],
                                            ident[0:20, 0:20])
                    idxs16 = small.tile([128, 160], i16, tag="idxs16")
                    nc.scalar.activation(
                        idxs16[0:16, :].rearrange("q (c pg) -> q pg c", pg=8),
                        psT2[:], AF.Copy)
                    nc.sync.dma_start(idxs16[16:32, :], idxs16[0:16, :])
                    nc.sync.dma_start(idxs16[32:64, :], idxs16[0:32, :])
                    nc.sync.dma_start(idxs16[64:128, :], idxs16[0:64, :])
                    # gather
                    dst = dstp.tile([128, K, CO], f32, tag="dst", name=f"dst{li}_{t}")
                    for off, n in GATHER_SPLITS:
                        nc.gpsimd.dma_gather(
                            dst[:, off // 128:(off + n) // 128, :], vtab[:],
                            idxs16[:, off // 16:(off + n) // 16], n, n, CO)
                    if dbg and li == 0 and t == 0:
                        nc.sync.dma_start(dbg_dst[:], dst[:])
                    return dst

                def back(t, dst):
                    tsl = slice(t * 128, (t + 1) * 128)
                    # bf16 casts (before in-place tree clobbers dst)
                    dbf = bfp.tile([128, K, CO], bf16, tag="dbf")
                    nc.scalar.activation(dbf[:], dst[:], AF.Copy)
                    dsq = bfp.tile([128, K, CO], bf16, tag="dbf")
                    nc.scalar.activation(dsq[:], dst[:], AF.Square)
                    # m = max_j dst (single strided reduce over j)
                    mloc = vcp.tile([128, CO], f32, tag="mloc")
                    nc.vector.tensor_reduce(mloc[:],
                                            dst[:].rearrange("p j c -> p c j"),
                                            mybir.AxisListType.X, AOT.max)
                    if dbg and li == 0:
                        nc.sync.dma_start(dbg_m[:, t * CO:(t + 1) * CO], mloc[:])
                    # stats matmuls (bf16)
                    if stage < 3:
                        return
                    for gi, (j0, gn) in enumerate(jgroups):
                        first = (t == 0 and gi == 0)
                        w = gn * CO
                        rhsv = dbf[:, j0:j0 + gn, :]
                        rhsq = dsq[:, j0:j0 + gn, :]
                        last = (t == NT - 1 and gi == len(jgroups) - 1)
                        nc.tensor.matmul(psS[:, 0:w], onesCol_bf[:], rhsv,
                                         start=first, stop=last,
                                         skip_group_check=True)
                        nc.tensor.matmul(psQ[:, 0:w], onesCol_bf[:], rhsq,
                                         start=first, stop=last,
                                         skip_group_check=True)
                        for h in range(NH):
                            nc.tensor.matmul(
                                psX[h][:, 0:w],
                                ubf_all[:, t, 128 * h:128 * h + CH], rhsv,
                                start=first, stop=last,
                                skip_group_check=True)
                    # pre-barrier y: wsum, transpose, stage into yPre / gmax
                    wsum = vcp.tile([128, CO], f32, tag="wsum")
                    nc.vector.tensor_tensor(out=wsum[:], in0=u_all[:, t, :],
                                            in1=mloc[:], op=AOT.add)
                    for h in range(NH):
                        psY = pyp.tile([128, 128], f32, tag="pyp")
                        nc.tensor.transpose(psY[0:CH, :],
                                            wsum[:, 128 * h:128 * h + CH],
                                            ident[:])
                        if not last_layer:
                            nc.scalar.activation(yPre[:, tsl], psY[0:CH, :],
                                                 AF.Copy)
                        else:
                            nc.vector.tensor_tensor(out=gmax[:, h, :],
                                                    in0=gmax[:, h, :],
                                                    in1=psY[0:CH, :], op=AOT.max)

                prev = front(0)
                for t in range(1, NT):
                    cur = front(t)
                    if prev is not None:
                        back(t - 1, prev)
                    prev = cur
                if prev is not None:
                    back(NT - 1, prev)

                # ---------- copy out S/Q, then Sum u / Sum u^2 in same slots ----
                if stage < 3:
                    continue
                rowS = rows.tile([1, 512], f32, tag="rowS")
                nc.scalar.activation(rowS[:], psS, AF.Copy)
                rowQ = rows.tile([1, 512], f32, tag="rowQ")
                nc.scalar.activation(rowQ[:], psQ, AF.Copy)

                # fold 512 -> CO in place
                for row in (rowS, rowQ, rowSu, rowQu):
                    wfull = 512
                    while wfull > CO:
                        half = wfull // 2
                        nc.vector.tensor_tensor(out=row[:, 0:half],
                                                in0=row[:, 0:half],
                                                in1=row[:, half:wfull], op=AOT.add)
                        wfull = half

                # cross-term: diag of psX via ttr with mask, then -> row
                junk = small.tile([128, 512], f32, tag="junk")
                crossRow = rows.tile([1, 256], f32, tag="crossRow")
                for h in range(NH):
                    ccol = small.tile([128, 1], f32, tag="ccol")
                    nc.vector.tensor_tensor(out=junk[0:CH, :], in0=psX[h][:],
                                            in1=masks[h][:], op=AOT.mult)
                    nc.vector.tensor_reduce(ccol[0:CH, :], junk[0:CH, :],
                                            mybir.AxisListType.X, AOT.add)
                    psCr = pmix.tile([1, CH], f32, tag="pmix")
                    nc.tensor.transpose(psCr[:], ccol[0:CH, :], ident[0:CH, 0:CH])
                    nc.scalar.activation(crossRow[:, 128 * h:128 * h + CH],
                                         psCr[:], AF.Copy)

                # ---------- per-core partial sums -> allreduce ----------
                statsrow = rows.tile([1, 512], f32, tag="statsrow")
                nc.vector.tensor_scalar(out=statsrow[:, 0:CO], in0=rowSu[:, 0:CO],
                                        scalar1=float(K), scalar2=None,
                                        op0=AOT.mult)
                nc.vector.tensor_tensor(out=statsrow[:, 0:CO],
                                        in0=statsrow[:, 0:CO],
                                        in1=rowS[:, 0:CO], op=AOT.add)
                nc.vector.tensor_scalar(out=statsrow[:, CO:2 * CO],
                                        in0=rowQu[:, 0:CO], scalar1=float(K),
                                        scalar2=None, op0=AOT.mult)
                nc.vector.tensor_scalar(out=crossRow[:, 0:CO], in0=crossRow[:, 0:CO],
                                        scalar1=2.0, scalar2=None, op0=AOT.mult)
                nc.vector.tensor_tensor(out=statsrow[:, CO:2 * CO],
                                        in0=statsrow[:, CO:2 * CO],
                                        in1=crossRow[:, 0:CO], op=AOT.add)
                nc.vector.tensor_tensor(out=statsrow[:, CO:2 * CO],
                                        in0=statsrow[:, CO:2 * CO],
                                        in1=rowQ[:, 0:CO], op=AOT.add)

                if stage < 4:
                    if dbg and li == 0:
                        nc.sync.dma_start(dbg_sr[:, 0:2 * CO], statsrow[:, 0:2 * CO])
                    continue
                ccin = dram.tile([1, 2 * CO], f32, tag=f"ccin{li}")
                ccout = dram.tile([1, 2 * CO], f32, tag=f"ccout{li}")
                nc.sync.dma_start(ccin[:], statsrow[:, 0:2 * CO])
                nc.gpsimd.collective_compute(
                    "AllReduce", AOT.add,
                    replica_groups=[list(range(NCORES))],
                    ins=[ccin.opt()], outs=[ccout.opt()])
                statsg = rows.tile([1, 512], f32, tag="statsg")
                nc.sync.dma_start(statsg[:, 0:2 * CO], ccout[:])
                if dbg and li == 0:
                    nc.sync.dma_start(dbg_sr[:, 0:2 * CO], statsrow[:, 0:2 * CO])
                    nc.sync.dma_start(dbg_sg[:, 0:2 * CO], statsg[:, 0:2 * CO])

                # ---------- BN scale/shift ----------
                cntr = 1.0 / float(B * N * K)
                meanR = rows.tile([1, 256], f32, tag="meanR")
                nc.vector.tensor_scalar(out=meanR[:, 0:CO], in0=statsg[:, 0:CO],
                                        scalar1=cntr, scalar2=None, op0=AOT.mult)
                t1R = rows.tile([1, 256], f32, tag="t1R")
                t2R = rows.tile([1, 256], f32, tag="t2R")
                # t1 = E[h^2] ; t2 = mean^2 ; t1 = var + eps
                nc.vector.tensor_scalar(out=t1R[:, 0:CO], in0=statsg[:, CO:2 * CO],
                                        scalar1=cntr, scalar2=None, op0=AOT.mult)
                nc.vector.tensor_tensor(out=t2R[:, 0:CO], in0=meanR[:, 0:CO],
                                        in1=meanR[:, 0:CO], op=AOT.mult)
                nc.vector.tensor_tensor(out=t1R[:, 0:CO], in0=t1R[:, 0:CO],
                                        in1=t2R[:, 0:CO], op=AOT.subtract)
                nc.vector.tensor_scalar(out=t1R[:, 0:CO], in0=t1R[:, 0:CO],
                                        scalar1=1e-5, scalar2=None, op0=AOT.add)
                # t2 = sqrt ; t1 = 1/sqrt ; scaleR(t2) = grow * t1
                nc.scalar.activation(t2R[:, 0:CO], t1R[:, 0:CO], AF.Sqrt)
                nc.vector.reciprocal(t1R[:, 0:CO], t2R[:, 0:CO])
                scaleR = rows.tile([1, 256], f32, tag="scaleR")
                nc.vector.tensor_tensor(out=scaleR[:, 0:CO], in0=grow[:].bitcast(f32),
                                        in1=t1R[:, 0:CO], op=AOT.mult)
                shiftR = rows.tile([1, 256], f32, tag="shiftR")
                nc.vector.tensor_tensor(out=shiftR[:, 0:CO], in0=meanR[:, 0:CO],
                                        in1=scaleR[:, 0:CO], op=AOT.mult)
                nc.vector.tensor_tensor(out=shiftR[:, 0:CO], in0=berow[:],
                                        in1=shiftR[:, 0:CO], op=AOT.subtract)

                scol = allp.tile([128, 2], f32, tag="scol")
                tcol = allp.tile([128, 2], f32, tag="tcol")
                for h in range(NH):
                    psc = pmix.tile([128, 1], f32, tag="pmix")
                    nc.tensor.transpose(psc[0:CH, :],
                                        scaleR[:, 128 * h:128 * h + CH],
                                        ident[0:1, 0:1])
                    nc.scalar.activation(scol[0:CH, h:h + 1], psc[0:CH, :], AF.Copy)
                    psc2 = pmix.tile([128, 1], f32, tag="pmix")
                    nc.tensor.transpose(psc2[0:CH, :],
                                        shiftR[:, 128 * h:128 * h + CH],
                                        ident[0:1, 0:1])
                    nc.scalar.activation(tcol[0:CH, h:h + 1], psc2[0:CH, :], AF.Copy)

                # ---------- y-phase (post-barrier): single affine+relu ----------
                if stage < 5:
                    continue
                if not last_layer:
                    yTn = ytp.tile([128, N], f32, tag=f"yt{(li + 1) % 2}")
                    nc.scalar.activation(yTn[0:CH, :], yPre[:, :], AF.Relu,
                                         bias=tcol[0:CH, 0:1],
                                         scale=scol[0:CH, 0:1])
                    yT = yTn
                    if dbg and li == 0:
                        nc.sync.dma_start(dbg_y[:], yTn[0:64, :])

            # ---------- head ----------
            if stage < 6:
                outSb_dummy = rows.tile([1, 256], f32, tag="crossRow")
                nc.vector.memset(outSb_dummy[:], 0.0)
                nc.sync.dma_start(out_ext[:], outSb_dummy[:])
            else:
                psH = pmix.tile([1, 256], f32, tag="pmix")
                for h in range(2):
                    gcol = small.tile([128, 1], f32, tag="ccol")
                    nc.vector.tensor_reduce(gcol[:], gmax[:, h, :],
                                            mybir.AxisListType.X, AOT.max)
                    nc.vector.tensor_scalar(out=gcol[:], in0=gcol[:],
                                            scalar1=scol[:, h:h + 1],
                                            scalar2=tcol[:, h:h + 1],
                                            op0=AOT.mult, op1=AOT.add)
                    nc.vector.tensor_scalar_max(gcol[:], gcol[:], 0.0)
                    nc.tensor.matmul(psH[:], gcol[:], woT_sb[:, h, :],
                                     start=(h == 0), stop=False,
                                     skip_group_check=True)
                nc.tensor.matmul(psH[:], onesRow[:, 0:1], boRow[:],
                                 start=False, stop=True, skip_group_check=True)
                outSb = rows.tile([1, 256], f32, tag="crossRow")
                nc.scalar.activation(outSb[:], psH[:], AF.Copy)
                nc.sync.dma_start(out_ext[:], outSb[:])

    nc.compile()
    return nc


def _host_prep(x, weights):
    """Build per-core input maps. x: [B, N, 3]."""
    shared = {}
    for li, (ci, co) in enumerate(LAYERS):
        W = np.asarray(weights[f"w{li + 1}"])            # [co, 2*ci]
        wc, wnn = W[:, :ci], W[:, ci:]
        shared[f"wcm{li}"] = np.ascontiguousarray((wc - wnn).T.astype(np.float32))
        shared[f"wn{li}"] = np.ascontiguousarray(wnn.T.astype(np.float32))
        shared[f"brow{li}"] = np.asarray(weights[f"b{li + 1}"]).reshape(1, co).astype(np.float32)
        shared[f"grow{li}"] = np.asarray(weights[f"g{li + 1}"]).reshape(1, co).astype(np.float32)
        shared[f"berow{li}"] = np.asarray(weights[f"be{li + 1}"]).reshape(1, co).astype(np.float32)
        G = 512 // co
        for h in range(-(-co // 128)):
            hc = min(128, co - 128 * h)
            mk = np.zeros((hc, 512), np.float32)
            for p in range(hc):
                for j in range(G):
                    mk[p, j * co + p + 128 * h] = 1.0
            shared[f"mask{li}_{h}"] = mk
    shared["ident"] = np.eye(128, dtype=np.float32)
    shared["onesr"] = np.ones((1, 128), dtype=np.float32)
    shared["woT"] = np.ascontiguousarray(np.asarray(weights["wo"]).T.astype(np.float32))
    shared["boRow"] = np.asarray(weights["bo"]).reshape(1, 256).astype(np.float32)
    ins = []
    for c in range(NCORES):
        m = dict(shared)
        m["xT"] = np.ascontiguousarray(np.asarray(x[c]).T.astype(np.float32))
        ins.append(m)
    return ins


def kernel(**inputs):
    from concourse.bass_utils import run_bass_kernel_spmd
    x = np.asarray(inputs["x"])
    if "nc" not in _BUILT:
        _BUILT["nc"] = _build()
    nc = _BUILT["nc"]
    in_maps = _host_prep(x, inputs)
    res = run_bass_kernel_spmd(nc, in_maps, list(range(NCORES))).results
    out = np.stack([res[c]["out"][0] for c in range(NCORES)], axis=0)
    return out.astype(np.float32)



# revision 8
# speedup vs baseline: 1.2863x; 1.2863x over previous
"""DynamicGraphCNN (DGCNN) forward pass on 8 Trainium2 NeuronCores.

Data-parallel over batch B=8: one point cloud per core. Per layer (edge-conv):
  scores  S'[i,j] = <x_i, x_j> - ||x_j||^2/2    (rank-equivalent to -dist^2)
  top-20 neighbors per row via DVE max/max_index/match_replace
  h[i,j] = u_i + v_{n(i,j)} with u = x(Wc-Wn)^T + b, v = x Wn^T
  BN (training stats over B,N,k) from global sums:
      Sum h   = 20*Sum u + Sum_{ij} v_n
      Sum h^2 = 20*Sum u^2 + 2*Sum_i u_i.s_i + Sum_{ij} v_n^2
  computed with bf16 PE matmuls over the gathered tiles (j-packed into psum),
  cross-term via u-weighted matmuls + diagonal-mask extraction.
  Cross-core reduction: one 8-core AllReduce per layer.
  y_i = relu(scale*(u_i + max_j v_n) + shift)   (monotone: max before affine)
Final: global max over points, then linear head.

v2: negxx fused into the score matmul via an augmented lhs row (L1/L2),
4-way SWDGE queue spread for the gathers, bf16 gather tables for L2/L3
(halving gather DMA and enabling 2x bf16 max-trees), stats matmuls grouped
by stationary operand to reuse loaded weights.
"""
import sys
sys.path.insert(0, '/opt/trn_rl_repo')

import numpy as np

B, N, K = 8, 2048, 20
NT = N // 128                      # 16 point tiles of 128
LAYERS = [(3, 64), (64, 128), (128, 256)]
NCORES = 8
GATHER_SPLITS = [(0, 640), (640, 640), (1280, 640), (1920, 640)]

_BUILT = {}


def _build(dbg=False):
    import contextlib
    import concourse.bacc as bacc
    import concourse.mybir as mybir
    import concourse.tile as tile

    f32 = mybir.dt.float32
    bf16 = mybir.dt.bfloat16
    i16 = mybir.dt.int16
    u32 = mybir.dt.uint32
    AOT = mybir.AluOpType
    AF = mybir.ActivationFunctionType

    nc = bacc.Bacc("TRN2", target_bir_lowering=False, debug=False,
                   num_devices=NCORES, num_swdge_queues=4)

    # ---------------- external tensors ----------------
    xT_in = nc.dram_tensor("xT", [3, N], f32, kind="ExternalInput")
    ext = {}
    for li, (ci, co) in enumerate(LAYERS):
        ext[f"wcm{li}"] = nc.dram_tensor(f"wcm{li}", [ci, co], f32, kind="ExternalInput")
        ext[f"wn{li}"] = nc.dram_tensor(f"wn{li}", [ci, co], f32, kind="ExternalInput")
        for rn in ("brow", "grow", "berow"):
            ext[f"{rn}{li}"] = nc.dram_tensor(f"{rn}{li}", [1, co], f32, kind="ExternalInput")
        for h in range(-(-co // 128)):
            hc = min(128, co - 128 * h)
            ext[f"mask{li}_{h}"] = nc.dram_tensor(
                f"mask{li}_{h}", [hc, 512], f32, kind="ExternalInput")
    ident_in = nc.dram_tensor("ident", [128, 128], f32, kind="ExternalInput")
    woT_in = nc.dram_tensor("woT", [256, 256], f32, kind="ExternalInput")
    bo_in = nc.dram_tensor("boRow", [1, 256], f32, kind="ExternalInput")
    out_ext = nc.dram_tensor("out", [1, 256], f32, kind="ExternalOutput")

    with tile.TileContext(nc) as tc:
        ctx = contextlib.ExitStack()
        with ctx:
            big = ctx.enter_context(tc.tile_pool(name="big", bufs=3))      # S / ysq
            ytp = ctx.enter_context(tc.tile_pool(name="ytp", bufs=1))      # yT (2 tags)
            allp = ctx.enter_context(tc.tile_pool(name="allp", bufs=1))    # layer residents
            resid = ctx.enter_context(tc.tile_pool(name="resid", bufs=1))  # constants
            dstp = ctx.enter_context(tc.tile_pool(name="dstp", bufs=2))
            bfp = ctx.enter_context(tc.tile_pool(name="bfp", bufs=2))      # dsq / trees
            small = ctx.enter_context(tc.tile_pool(name="small", bufs=2))  # idx plumbing
            rows = ctx.enter_context(tc.tile_pool(name="rows", bufs=1))    # [1,*] rows
            vcp = ctx.enter_context(tc.tile_pool(name="vcp", bufs=2))      # staging
            dram = ctx.enter_context(tc.tile_pool(name="dram", bufs=1, space="DRAM"))
            pscore = ctx.enter_context(tc.tile_pool(name="pscore", bufs=2, space="PSUM"))
            pyp = ctx.enter_context(tc.tile_pool(name="pyp", bufs=1, space="PSUM"))
            pmix = ctx.enter_context(tc.tile_pool(name="pmix", bufs=2, space="PSUM"))
            pstat = ctx.enter_context(tc.tile_pool(name="pstat", bufs=1, space="PSUM"))

            # ---------- kernel-lifetime constants ----------
            ident = resid.tile([128, 128], f32, tag="ident")
            nc.sync.dma_start(ident[:], ident_in[:])
            onesRow = resid.tile([1, 128], f32, tag="onesRow")
            nc.vector.memset(onesRow[:], 1.0)
            onesColF = resid.tile([128, 1], f32, tag="onesColF")
            nc.vector.memset(onesColF[:], 1.0)
            onesCol_bf = resid.tile([128, 1], bf16, tag="onesColbf")
            nc.vector.memset(onesCol_bf[:], 1.0)
            negHalfCol = resid.tile([128, 1], f32, tag="negHalfCol")
            nc.vector.memset(negHalfCol[:], -0.5)
            woT_sb = resid.tile([128, 2, 256], f32, tag="woT")
            for h in range(2):
                nc.sync.dma_start(woT_sb[:, h, :], woT_in[128 * h:128 * (h + 1), :])
            boRow = resid.tile([1, 256], f32, tag="boRow")
            nc.sync.dma_start(boRow[:], bo_in[:])
            gmax = resid.tile([128, 2, 128], f32, tag="gmax")
            nc.vector.memset(gmax[:], -1e30)

            # yT carries the CI feature rows plus one negxx row for L1/L2 so
            # the -||x_j||^2/2 bias rides the score matmul as an extra
            # contraction row. Engine accesses need 32-aligned base
            # partitions, so L1 (CI=3) pads rows 3..31 with zeros and puts
            # negxx at row 32; L2 (CI=64) puts it at row 64. L3 (CI=128) has
            # no spare partition and keeps the separate bias matmul.
            yT = ytp.tile([128, N], f32, tag="yt0")
            nc.vector.memset(yT[0:33, :], 0.0)
            nc.sync.dma_start(yT[0:3, :], xT_in[:])

            for li, (CI, CO) in enumerate(LAYERS):
                NH = -(-CO // 128)
                CH = min(128, CO)
                G = 512 // CO
                jgroups = []
                j0 = 0
                while j0 < K:
                    jgroups.append((j0, min(G, K - j0)))
                    j0 += G
                last_layer = (li == len(LAYERS) - 1)
                # fused negxx: contraction rows 0..CIa-1 are features (+ zero
                # padding), row CIa is negxx; CIa must be 32-aligned.
                CIa = -32 * (-CI // 32)
                aug_scores = (CIa + 1 <= 128)
                gdt = bf16 if (CO * 2) % 256 == 0 else f32   # gather dtype

                # ---------- weights / rows ----------
                wcm = allp.tile([CI, CO], f32, tag="wcm")
                nc.sync.dma_start(wcm[:], ext[f"wcm{li}"][:])
                wn = allp.tile([CI, CO], f32, tag="wn")
                nc.sync.dma_start(wn[:], ext[f"wn{li}"][:])
                brow = allp.tile([1, CO], f32, tag="brow")
                nc.sync.dma_start(brow[:], ext[f"brow{li}"][:])
                grow = allp.tile([1, CO], f32, tag="grow")
                nc.sync.dma_start(grow[:], ext[f"grow{li}"][:])
                berow = allp.tile([1, CO], f32, tag="berow")
                nc.sync.dma_start(berow[:], ext[f"berow{li}"][:])
                masks = []
                for h in range(NH):
                    mk = allp.tile([CH, 512], f32, tag=f"mask{h}")
                    nc.sync.dma_start(mk[:], ext[f"mask{li}_{h}"][:])
                    masks.append(mk)

                # ---------- prep: negxx row via PE ----------
                ysq = big.tile([128, N], f32, tag="big")
                nc.scalar.activation(ysq[0:CI, :], yT[0:CI, :], AF.Square)
                if aug_scores:
                    negxx = yT[CIa:CIa + 1, :]
                else:
                    negxx3 = allp.tile([1, N], f32, tag="negxx")
                    negxx = negxx3[:]
                for nj in range(4):
                    ps = pmix.tile([1, 512], f32, tag="pmix")
                    nc.tensor.matmul(ps[:], negHalfCol[0:CI, :],
                                     ysq[0:CI, nj * 512:(nj + 1) * 512],
                                     start=True, stop=True)
                    nc.scalar.activation(negxx[:, nj * 512:(nj + 1) * 512],
                                         ps[:], AF.Copy)

                # ---------- prep: u, v per tile; v -> vtab ----------
                vtab = dram.tile([N, CO], gdt, tag=f"vtab{li}")
                u_all = allp.tile([128, NT, CO], f32, tag="u_all")
                ubf_all = allp.tile([128, NT, CO], bf16, tag="ubf_all")
                for t in range(NT):
                    tsl = slice(t * 128, (t + 1) * 128)
                    psU = pmix.tile([128, CO], f32, tag="pmix")
                    nc.tensor.matmul(psU[:], yT[0:CI, tsl], wcm[:], start=True, stop=False)
                    nc.tensor.matmul(psU[:], onesRow[:, 0:128], brow[:],
                                     start=False, stop=True)
                    nc.scalar.activation(u_all[:, t, :], psU[:], AF.Copy)
                    nc.scalar.activation(ubf_all[:, t, :], psU[:], AF.Copy)
                    psV = pmix.tile([128, CO], f32, tag="pmix")
                    nc.tensor.matmul(psV[:], yT[0:CI, tsl], wn[:],
                                     start=True, stop=True)
                    vst = vcp.tile([128, CO], gdt, tag="vst")
                    nc.scalar.activation(vst[:], psV[:], AF.Copy)
                    nc.sync.dma_start(vtab[tsl, :], vst[:])

                # ---------- Sum u / Sum u^2 at prep (fp32, exact) ----------
                psSQu = pstat.tile([33, 512], f32, tag="psSQ", name="psSQu")
                psSu = psSQu[0:1, :]
                psQu = psSQu[32:33, :]
                u_flat = u_all[:].rearrange("p t c -> p (t c)")
                nuv = NT * CO // 512
                for s in range(nuv):
                    usqf = vcp.tile([128, 512], f32, tag="usq")
                    nc.scalar.activation(usqf[:], u_flat[:, 512 * s:512 * (s + 1)],
                                         AF.Square)
                    nc.tensor.matmul(psSu, onesColF[:],
                                     u_flat[:, 512 * s:512 * (s + 1)],
                                     start=(s == 0), stop=(s == nuv - 1),
                                     skip_group_check=True)
                    nc.tensor.matmul(psQu, onesColF[:], usqf[:],
                                     start=(s == 0), stop=(s == nuv - 1),
                                     skip_group_check=True)
                rowSu = rows.tile([1, 512], f32, tag="rowSu")
                nc.scalar.activation(rowSu[:], psSu, AF.Copy)
                rowQu = rows.tile([1, 512], f32, tag="rowQu")
                nc.scalar.activation(rowQu[:], psQu, AF.Copy)

                # ---------- stat psums (locked for the layer) ----------
                psSQ = pstat.tile([33, 512], f32, tag="psSQ", name="psSQm")
                psS = psSQ[0:1, :]
                psQ = psSQ[32:33, :]
                psX = [pstat.tile([CH, 512], f32, tag=f"psX{h}", name=f"psX{li}_{h}") for h in range(NH)]

                yPre = None
                if not last_layer:
                    yPre = big.tile([CH, N], f32, tag="big", name=f"yPre{li}")

                # ---------- main loop: 1-tile software pipeline ----------
                def front(t):
                    tsl = slice(t * 128, (t + 1) * 128)
                    S = big.tile([128, N], f32, tag="big", name=f"S{li}_{t}")
                    if aug_scores:
                        aug = small.tile([CIa + 1, 128], f32, tag=f"aug{li}",
                                         name=f"aug{li}_{t}")
                        nc.scalar.activation(aug[0:CIa, :], yT[0:CIa, tsl], AF.Copy)
                        nc.vector.memset(aug[CIa:CIa + 1, :], 1.0)
                    for nj in range(4):
                        psSc = pscore.tile([128, 512], f32, tag="psc")
                        if aug_scores:
                            nc.tensor.matmul(psSc[:], aug[:],
                                             yT[0:CIa + 1, nj * 512:(nj + 1) * 512],
                                             start=True, stop=True)
                        else:
                            nc.tensor.matmul(psSc[:], yT[0:CI, tsl],
                                             yT[0:CI, nj * 512:(nj + 1) * 512],
                                             start=True, stop=False)
                            nc.tensor.matmul(psSc[:], onesRow[:, 0:128],
                                             negxx[:, nj * 512:(nj + 1) * 512],
                                             start=False, stop=True)
                        nc.scalar.activation(S[:, nj * 512:(nj + 1) * 512],
                                             psSc[:], AF.Copy)
                    # top-20: 3 rounds of 8
                    maxv = small.tile([128, 24], f32, tag="maxv")
                    idxu = small.tile([128, 24], u32, tag="idxu")
                    for r in range(3):
                        rs = slice(8 * r, 8 * (r + 1))
                        nc.vector.max(maxv[:, rs], S[:])
                        nc.vector.max_index(idxu[:, rs], maxv[:, rs], S[:])
                        if r < 2:
                            nc.vector.match_replace(S[:], maxv[:, rs], S[:], -1e30)
                    # index plumbing: [128,20] u32 -> wrapped [128,160] i16
                    idxf = small.tile([128, 20], f32, tag="idxf")
                    nc.vector.tensor_copy(idxf[:], idxu[:, 0:20])
                    psT1 = pmix.tile([20, 128], f32, tag="pmix")
                    nc.tensor.transpose(psT1[:], idxf[:], ident[:])
                    idxT = small.tile([20, 128], f32, tag="idxT")
                    nc.scalar.activation(idxT[:], psT1[:], AF.Copy)
                    psT2 = pmix.tile([16, 8, 20], f32, tag="pmix")
                    for pg in range(8):
                        nc.tensor.transpose(psT2[:, pg, :],
                                            idxT[:, pg * 16:(pg + 1) * 16],
                                            ident[0:20, 0:20])
                    idxs16 = small.tile([128, 160], i16, tag="idxs16")
                    nc.scalar.activation(
                        idxs16[0:16, :].rearrange("q (c pg) -> q pg c", pg=8),
                        psT2[:], AF.Copy)
                    nc.sync.dma_start(idxs16[16:32, :], idxs16[0:16, :])
                    nc.sync.dma_start(idxs16[32:64, :], idxs16[0:32, :])
                    nc.sync.dma_start(idxs16[64:128, :], idxs16[0:64, :])
                    # gather (4-way SWDGE queue spread)
                    dst = dstp.tile([128, K, CO], gdt, tag="dst", name=f"dst{li}_{t}")
                    for qi, (off, n) in enumerate(GATHER_SPLITS):
                        nc.gpsimd.dma_gather(
                            dst[:, off // 128:(off + n) // 128, :], vtab[:],
                            idxs16[:, off // 16:(off + n) // 16], n, n, CO,
                            queue_num=qi)
                    return dst

                def back(t, dst):
                    tsl = slice(t * 128, (t + 1) * 128)
                    if gdt is bf16:
                        dbf = dst
                        dsq = bfp.tile([128, K, CO], bf16, tag="dsq")
                        nc.scalar.activation(dsq[:], dst[:], AF.Square)
                        # m = max_j dst: 2x bf16 max tree 20->10->5->(2,2,1)->1
                        mt1 = bfp.tile([128, 10, CO], bf16, tag="mt1")
                        nc.vector.tensor_tensor(out=mt1[:], in0=dst[:, 0:10, :],
                                                in1=dst[:, 10:20, :], op=AOT.max)
                        mt2 = vcp.tile([128, 5, CO], bf16, tag="mt2")
                        nc.vector.tensor_tensor(out=mt2[:], in0=mt1[:, 0:5, :],
                                                in1=mt1[:, 5:10, :], op=AOT.max)
                        nc.vector.tensor_tensor(out=mt2[:, 0:2, :],
                                                in0=mt2[:, 0:2, :],
                                                in1=mt2[:, 2:4, :], op=AOT.max)
                        nc.vector.tensor_tensor(out=mt2[:, 0:1, :],
                                                in0=mt2[:, 0:1, :],
                                                in1=mt2[:, 1:2, :], op=AOT.max)
                        mloc = vcp.tile([128, CO], f32, tag="mloc")
                        nc.vector.tensor_tensor(out=mloc[:],
                                                in0=mt2[:, 0:1, :],
                                                in1=mt2[:, 4:5, :], op=AOT.max)
                    else:
                        dbf = bfp.tile([128, K, CO], bf16, tag="dbf")
                        nc.scalar.activation(dbf[:], dst[:], AF.Copy)
                        dsq = bfp.tile([128, K, CO], bf16, tag="dsq")
                        nc.scalar.activation(dsq[:], dst[:], AF.Square)
                        mloc = vcp.tile([128, CO], f32, tag="mloc")
                        nc.vector.tensor_reduce(mloc[:],
                                                dst[:].rearrange("p j c -> p c j"),
                                                mybir.AxisListType.X, AOT.max)
                    # stats matmuls (bf16), grouped by stationary operand
                    first = (t == 0)
                    last = (t == NT - 1)
                    for gi, (j0, gn) in enumerate(jgroups):
                        w = gn * CO
                        nc.tensor.matmul(psS[:, 0:w], onesCol_bf[:],
                                         dbf[:, j0:j0 + gn, :],
                                         start=(first and gi == 0),
                                         stop=(last and gi == len(jgroups) - 1),
                                         skip_group_check=True)
                    for gi, (j0, gn) in enumerate(jgroups):
                        w = gn * CO
                        nc.tensor.matmul(psQ[:, 0:w], onesCol_bf[:],
                                         dsq[:, j0:j0 + gn, :],
                                         start=(first and gi == 0),
                                         stop=(last and gi == len(jgroups) - 1),
                                         skip_group_check=True)
                    for h in range(NH):
                        for gi, (j0, gn) in enumerate(jgroups):
                            w = gn * CO
                            nc.tensor.matmul(
                                psX[h][:, 0:w],
                                ubf_all[:, t, 128 * h:128 * h + CH],
                                dbf[:, j0:j0 + gn, :],
                                start=(first and gi == 0),
                                stop=(last and gi == len(jgroups) - 1),
                                skip_group_check=True)
                    # pre-barrier y: wsum, transpose, stage into yPre / gmax
                    wsum = vcp.tile([128, CO], f32, tag="wsum")
                    nc.vector.tensor_tensor(out=wsum[:], in0=u_all[:, t, :],
                                            in1=mloc[:], op=AOT.add)
                    for h in range(NH):
                        psY = pyp.tile([128, 128], f32, tag="pyp")
                        nc.tensor.transpose(psY[0:CH, :],
                                            wsum[:, 128 * h:128 * h + CH],
                                            ident[:])
                        if not last_layer:
                            nc.scalar.activation(yPre[:, tsl], psY[0:CH, :],
                                                 AF.Copy)
                        else:
                            nc.vector.tensor_tensor(out=gmax[:, h, :],
                                                    in0=gmax[:, h, :],
                                                    in1=psY[0:CH, :], op=AOT.max)

                prev = front(0)
                for t in range(1, NT):
                    cur = front(t)
                    back(t - 1, prev)
                    prev = cur
                back(NT - 1, prev)

                # ---------- copy out S/Q, then fold 512 -> CO ----------
                rowS = rows.tile([1, 512], f32, tag="rowS")
                nc.scalar.activation(rowS[:], psS, AF.Copy)
                rowQ = rows.tile([1, 512], f32, tag="rowQ")
                nc.scalar.activation(rowQ[:], psQ, AF.Copy)

                for row in (rowS, rowQ, rowSu, rowQu):
                    wfull = 512
                    while wfull > CO:
                        half = wfull // 2
                        nc.vector.tensor_tensor(out=row[:, 0:half],
                                                in0=row[:, 0:half],
                                                in1=row[:, half:wfull], op=AOT.add)
                        wfull = half

                # cross-term: diag of psX via ttr with mask, then -> row
                junk = small.tile([128, 512], f32, tag="junk")
                crossRow = rows.tile([1, 256], f32, tag="crossRow")
                for h in range(NH):
                    ccol = small.tile([128, 1], f32, tag="ccol")
                    nc.vector.tensor_tensor(out=junk[0:CH, :], in0=psX[h][:],
                                            in1=masks[h][:], op=AOT.mult)
                    nc.vector.tensor_reduce(ccol[0:CH, :], junk[0:CH, :],
                                            mybir.AxisListType.X, AOT.add)
                    psCr = pmix.tile([1, CH], f32, tag="pmix")
                    nc.tensor.transpose(psCr[:], ccol[0:CH, :], ident[0:CH, 0:CH])
                    nc.scalar.activation(crossRow[:, 128 * h:128 * h + CH],
                                         psCr[:], AF.Copy)

                # ---------- per-core partial sums -> allreduce ----------
                statsrow = rows.tile([1, 512], f32, tag="statsrow")
                nc.vector.tensor_scalar(out=statsrow[:, 0:CO], in0=rowSu[:, 0:CO],
                                        scalar1=float(K), scalar2=None,
                                        op0=AOT.mult)
                nc.vector.tensor_tensor(out=statsrow[:, 0:CO],
                                        in0=statsrow[:, 0:CO],
                                        in1=rowS[:, 0:CO], op=AOT.add)
                nc.vector.tensor_scalar(out=statsrow[:, CO:2 * CO],
                                        in0=rowQu[:, 0:CO], scalar1=float(K),
                                        scalar2=None, op0=AOT.mult)
                nc.vector.tensor_scalar(out=crossRow[:, 0:CO], in0=crossRow[:, 0:CO],
                                        scalar1=2.0, scalar2=None, op0=AOT.mult)
                nc.vector.tensor_tensor(out=statsrow[:, CO:2 * CO],
                                        in0=statsrow[:, CO:2 * CO],
                                        in1=crossRow[:, 0:CO], op=AOT.add)
                nc.vector.tensor_tensor(out=statsrow[:, CO:2 * CO],
                                        in0=statsrow[:, CO:2 * CO],
                                        in1=rowQ[:, 0:CO], op=AOT.add)

                ccin = dram.tile([1, 2 * CO], f32, tag=f"ccin{li}")
                ccout = dram.tile([1, 2 * CO], f32, tag=f"ccout{li}")
                nc.sync.dma_start(ccin[:], statsrow[:, 0:2 * CO])
                nc.gpsimd.collective_compute(
                    "AllReduce", AOT.add,
                    replica_groups=[list(range(NCORES))],
                    ins=[ccin.opt()], outs=[ccout.opt()])
                statsg = rows.tile([1, 512], f32, tag="statsg")
                nc.sync.dma_start(statsg[:, 0:2 * CO], ccout[:])

                # ---------- BN scale/shift ----------
                cntr = 1.0 / float(B * N * K)
                meanR = rows.tile([1, 256], f32, tag="meanR")
                nc.vector.tensor_scalar(out=meanR[:, 0:CO], in0=statsg[:, 0:CO],
                                        scalar1=cntr, scalar2=None, op0=AOT.mult)
                t1R = rows.tile([1, 256], f32, tag="t1R")
                t2R = rows.tile([1, 256], f32, tag="t2R")
                nc.vector.tensor_scalar(out=t1R[:, 0:CO], in0=statsg[:, CO:2 * CO],
                                        scalar1=cntr, scalar2=None, op0=AOT.mult)
                nc.vector.tensor_tensor(out=t2R[:, 0:CO], in0=meanR[:, 0:CO],
                                        in1=meanR[:, 0:CO], op=AOT.mult)
                nc.vector.tensor_tensor(out=t1R[:, 0:CO], in0=t1R[:, 0:CO],
                                        in1=t2R[:, 0:CO], op=AOT.subtract)
                nc.vector.tensor_scalar(out=t1R[:, 0:CO], in0=t1R[:, 0:CO],
                                        scalar1=1e-5, scalar2=None, op0=AOT.add)
                nc.scalar.activation(t2R[:, 0:CO], t1R[:, 0:CO], AF.Sqrt)
                nc.vector.reciprocal(t1R[:, 0:CO], t2R[:, 0:CO])
                scaleR = rows.tile([1, 256], f32, tag="scaleR")
                nc.vector.tensor_tensor(out=scaleR[:, 0:CO], in0=grow[:].bitcast(f32),
                                        in1=t1R[:, 0:CO], op=AOT.mult)
                shiftR = rows.tile([1, 256], f32, tag="shiftR")
                nc.vector.tensor_tensor(out=shiftR[:, 0:CO], in0=meanR[:, 0:CO],
                                        in1=scaleR[:, 0:CO], op=AOT.mult)
                nc.vector.tensor_tensor(out=shiftR[:, 0:CO], in0=berow[:],
                                        in1=shiftR[:, 0:CO], op=AOT.subtract)

                scol = allp.tile([128, 2], f32, tag="scol")
                tcol = allp.tile([128, 2], f32, tag="tcol")
                for h in range(NH):
                    psc = pmix.tile([128, 1], f32, tag="pmix")
                    nc.tensor.transpose(psc[0:CH, :],
                                        scaleR[:, 128 * h:128 * h + CH],
                                        ident[0:1, 0:1])
                    nc.scalar.activation(scol[0:CH, h:h + 1], psc[0:CH, :], AF.Copy)
                    psc2 = pmix.tile([128, 1], f32, tag="pmix")
                    nc.tensor.transpose(psc2[0:CH, :],
                                        shiftR[:, 128 * h:128 * h + CH],
                                        ident[0:1, 0:1])
                    nc.scalar.activation(tcol[0:CH, h:h + 1], psc2[0:CH, :], AF.Copy)

                # ---------- y-phase (post-barrier): single affine+relu ----------
                if not last_layer:
                    yTn = ytp.tile([128, N], f32, tag=f"yt{(li + 1) % 2}")
                    nc.scalar.activation(yTn[0:CH, :], yPre[:, :], AF.Relu,
                                         bias=tcol[0:CH, 0:1],
                                         scale=scol[0:CH, 0:1])
                    yT = yTn

            # ---------- head ----------
            psH = pmix.tile([1, 256], f32, tag="pmix")
            for h in range(2):
                gcol = small.tile([128, 1], f32, tag="ccol")
                nc.vector.tensor_reduce(gcol[:], gmax[:, h, :],
                                        mybir.AxisListType.X, AOT.max)
                nc.vector.tensor_scalar(out=gcol[:], in0=gcol[:],
                                        scalar1=scol[:, h:h + 1],
                                        scalar2=tcol[:, h:h + 1],
                                        op0=AOT.mult, op1=AOT.add)
                nc.vector.tensor_scalar_max(gcol[:], gcol[:], 0.0)
                nc.tensor.matmul(psH[:], gcol[:], woT_sb[:, h, :],
                                 start=(h == 0), stop=False,
                                 skip_group_check=True)
            nc.tensor.matmul(psH[:], onesRow[:, 0:1], boRow[:],
                             start=False, stop=True, skip_group_check=True)
            outSb = rows.tile([1, 256], f32, tag="crossRow")
            nc.scalar.activation(outSb[:], psH[:], AF.Copy)
            nc.sync.dma_start(out_ext[:], outSb[:])

    nc.compile()
    return nc


def _host_prep(x, weights):
    """Build per-core input maps. x: [B, N, 3]."""
    shared = {}
    for li, (ci, co) in enumerate(LAYERS):
        W = np.asarray(weights[f"w{li + 1}"])            # [co, 2*ci]
        wc, wnn = W[:, :ci], W[:, ci:]
        shared[f"wcm{li}"] = np.ascontiguousarray((wc - wnn).T.astype(np.float32))
        shared[f"wn{li}"] = np.ascontiguousarray(wnn.T.astype(np.float32))
        shared[f"brow{li}"] = np.asarray(weights[f"b{li + 1}"]).reshape(1, co).astype(np.float32)
        shared[f"grow{li}"] = np.asarray(weights[f"g{li + 1}"]).reshape(1, co).astype(np.float32)
        shared[f"berow{li}"] = np.asarray(weights[f"be{li + 1}"]).reshape(1, co).astype(np.float32)
        G = 512 // co
        for h in range(-(-co // 128)):
            hc = min(128, co - 128 * h)
            mk = np.zeros((hc, 512), np.float32)
            for p in range(hc):
                for j in range(G):
                    mk[p, j * co + p + 128 * h] = 1.0
            shared[f"mask{li}_{h}"] = mk
    shared["ident"] = np.eye(128, dtype=np.float32)
    shared["woT"] = np.ascontiguousarray(np.asarray(weights["wo"]).T.astype(np.float32))
    shared["boRow"] = np.asarray(weights["bo"]).reshape(1, 256).astype(np.float32)
    ins = []
    for c in range(NCORES):
        m = dict(shared)
        m["xT"] = np.ascontiguousarray(np.asarray(x[c]).T.astype(np.float32))
        ins.append(m)
    return ins


def kernel(**inputs):
    from concourse.bass_utils import run_bass_kernel_spmd
    x = np.asarray(inputs["x"])
    if "nc" not in _BUILT:
        _BUILT["nc"] = _build()
    nc = _BUILT["nc"]
    in_maps = _host_prep(x, inputs)
    res = run_bass_kernel_spmd(nc, in_maps, list(range(NCORES))).results
    out = np.stack([res[c]["out"][0] for c in range(NCORES)], axis=0)
    return out.astype(np.float32)


# revision 12
# speedup vs baseline: 1.4632x; 1.1375x over previous
"""DynamicGraphCNN (DGCNN) forward pass on 8 Trainium2 NeuronCores.

Data-parallel over batch B=8: one point cloud per core. Per layer (edge-conv):
  scores  S'[i,j] = <x_i, x_j> - ||x_j||^2/2    (rank-equivalent to -dist^2)
  top-20 neighbors per row via DVE max/max_index/match_replace
  h[i,j] = u_i + v_{n(i,j)} with u = x(Wc-Wn)^T + b, v = x Wn^T
  BN (training stats over B,N,k) from global sums:
      Sum h   = 20*Sum u + Sum_{ij} v_n
      Sum h^2 = 20*Sum u^2 + 2*Sum_i u_i.s_i + Sum_{ij} v_n^2
  computed with bf16 PE matmuls over the gathered tiles (j-packed into psum),
  cross-term via u-weighted matmuls + diagonal-mask extraction.
  Cross-core reduction: one 8-core AllReduce per layer.
  y_i = relu(scale*(u_i + max_j v_n) + shift)   (monotone: max before affine)
Final: global max over points, then linear head.

v2: negxx fused into the score matmul via an augmented lhs row (L1/L2),
4-way SWDGE queue spread for the gathers, bf16 gather tables for L2/L3
(halving gather DMA and enabling 2x bf16 max-trees), stats matmuls grouped
by stationary operand to reuse loaded weights.
"""
import sys
sys.path.insert(0, '/opt/trn_rl_repo')

import numpy as np

B, N, K = 8, 2048, 20
NT = N // 128                      # 16 point tiles of 128
LAYERS = [(3, 64), (64, 128), (128, 256)]
NCORES = 8
GATHER_SPLITS = [(0, 640), (640, 640), (1280, 640), (1920, 640)]

_BUILT = {}


def _build(dbg=False):
    import contextlib
    import concourse.bacc as bacc
    import concourse.mybir as mybir
    import concourse.tile as tile

    f32 = mybir.dt.float32
    bf16 = mybir.dt.bfloat16
    i16 = mybir.dt.int16
    u32 = mybir.dt.uint32
    AOT = mybir.AluOpType
    AF = mybir.ActivationFunctionType

    nc = bacc.Bacc("TRN2", target_bir_lowering=False, debug=False,
                   num_devices=NCORES, num_swdge_queues=4)

    # ---------------- external tensors ----------------
    xT_in = nc.dram_tensor("xT", [3, N], f32, kind="ExternalInput")
    ext = {}
    for li, (ci, co) in enumerate(LAYERS):
        ext[f"wcm{li}"] = nc.dram_tensor(f"wcm{li}", [ci, co], f32, kind="ExternalInput")
        ext[f"wn{li}"] = nc.dram_tensor(f"wn{li}", [ci, co], f32, kind="ExternalInput")
        for rn in ("brow", "grow", "berow"):
            ext[f"{rn}{li}"] = nc.dram_tensor(f"{rn}{li}", [1, co], f32, kind="ExternalInput")
        for h in range(-(-co // 128)):
            hc = min(128, co - 128 * h)
            ext[f"mask{li}_{h}"] = nc.dram_tensor(
                f"mask{li}_{h}", [hc, 512], f32, kind="ExternalInput")
    ident_in = nc.dram_tensor("ident", [128, 128], f32, kind="ExternalInput")
    woT_in = nc.dram_tensor("woT", [256, 256], f32, kind="ExternalInput")
    bo_in = nc.dram_tensor("boRow", [1, 256], f32, kind="ExternalInput")
    out_ext = nc.dram_tensor("out", [1, 256], f32, kind="ExternalOutput")

    with tile.TileContext(nc) as tc:
        ctx = contextlib.ExitStack()
        with ctx:
            big = ctx.enter_context(tc.tile_pool(name="big", bufs=3))      # S / ysq
            ytp = ctx.enter_context(tc.tile_pool(name="ytp", bufs=1))      # yT (2 tags)
            allp = ctx.enter_context(tc.tile_pool(name="allp", bufs=1))    # layer residents
            resid = ctx.enter_context(tc.tile_pool(name="resid", bufs=1))  # constants
            dstp = ctx.enter_context(tc.tile_pool(name="dstp", bufs=2))
            bfp = ctx.enter_context(tc.tile_pool(name="bfp", bufs=2))      # dsq / trees
            small = ctx.enter_context(tc.tile_pool(name="small", bufs=2))  # idx plumbing
            rows = ctx.enter_context(tc.tile_pool(name="rows", bufs=1))    # [1,*] rows
            vcp = ctx.enter_context(tc.tile_pool(name="vcp", bufs=2))      # staging
            dram = ctx.enter_context(tc.tile_pool(name="dram", bufs=1, space="DRAM"))
            pscore = ctx.enter_context(tc.tile_pool(name="pscore", bufs=2, space="PSUM"))
            pyp = ctx.enter_context(tc.tile_pool(name="pyp", bufs=1, space="PSUM"))
            pmix = ctx.enter_context(tc.tile_pool(name="pmix", bufs=2, space="PSUM"))
            pstat = ctx.enter_context(tc.tile_pool(name="pstat", bufs=1, space="PSUM"))

            # ---------- kernel-lifetime constants ----------
            ident = resid.tile([128, 128], f32, tag="ident")
            nc.sync.dma_start(ident[:], ident_in[:])
            onesRow = resid.tile([1, 128], f32, tag="onesRow")
            nc.vector.memset(onesRow[:], 1.0)
            onesColF = resid.tile([128, 1], f32, tag="onesColF")
            nc.vector.memset(onesColF[:], 1.0)
            onesCol_bf = resid.tile([128, 1], bf16, tag="onesColbf")
            nc.vector.memset(onesCol_bf[:], 1.0)
            negHalfCol = resid.tile([128, 1], f32, tag="negHalfCol")
            nc.vector.memset(negHalfCol[:], -0.5)
            woT_sb = resid.tile([128, 2, 256], f32, tag="woT")
            for h in range(2):
                nc.sync.dma_start(woT_sb[:, h, :], woT_in[128 * h:128 * (h + 1), :])
            boRow = resid.tile([1, 256], f32, tag="boRow")
            nc.sync.dma_start(boRow[:], bo_in[:])
            gmax = resid.tile([128, 2, 128], f32, tag="gmax")
            nc.vector.memset(gmax[:], -1e30)

            # yT carries the CI feature rows plus one negxx row for L1/L2 so
            # the -||x_j||^2/2 bias rides the score matmul as an extra
            # contraction row. Engine accesses need 32-aligned base
            # partitions, so L1 (CI=3) pads rows 3..31 with zeros and puts
            # negxx at row 32; L2 (CI=64) puts it at row 64. L3 (CI=128) has
            # no spare partition and keeps the separate bias matmul.
            yT = ytp.tile([128, N], f32, tag="yt0")
            nc.vector.memset(yT[0:33, :], 0.0)
            nc.sync.dma_start(yT[0:3, :], xT_in[:])

            for li, (CI, CO) in enumerate(LAYERS):
                NH = -(-CO // 128)
                CH = min(128, CO)
                G = 512 // CO
                jgroups = []
                j0 = 0
                while j0 < K:
                    jgroups.append((j0, min(G, K - j0)))
                    j0 += G
                last_layer = (li == len(LAYERS) - 1)
                # fused negxx: contraction rows 0..CIa-1 are features (+ zero
                # padding), row CIa is negxx; CIa must be 32-aligned.
                CIa = -32 * (-CI // 32)
                aug_scores = (CIa + 1 <= 128)
                gdt = bf16 if (CO * 2) % 256 == 0 else f32   # gather dtype

                # ---------- weights / rows ----------
                wcm = allp.tile([CI, CO], f32, tag="wcm")
                nc.sync.dma_start(wcm[:], ext[f"wcm{li}"][:])
                wn = allp.tile([CI, CO], f32, tag="wn")
                nc.sync.dma_start(wn[:], ext[f"wn{li}"][:])
                brow = allp.tile([1, CO], f32, tag="brow")
                nc.sync.dma_start(brow[:], ext[f"brow{li}"][:])
                grow = allp.tile([1, CO], f32, tag="grow")
                nc.sync.dma_start(grow[:], ext[f"grow{li}"][:])
                berow = allp.tile([1, CO], f32, tag="berow")
                nc.sync.dma_start(berow[:], ext[f"berow{li}"][:])
                masks = []
                for h in range(NH):
                    mk = allp.tile([CH, 512], f32, tag=f"mask{h}")
                    nc.sync.dma_start(mk[:], ext[f"mask{li}_{h}"][:])
                    masks.append(mk)

                # ---------- prep: negxx row via PE ----------
                ysq = big.tile([128, N], f32, tag="big")
                nc.scalar.activation(ysq[0:CI, :], yT[0:CI, :], AF.Square)
                if aug_scores:
                    negxx = yT[CIa:CIa + 1, :]
                else:
                    negxx3 = allp.tile([1, N], f32, tag="negxx")
                    negxx = negxx3[:]
                for nj in range(4):
                    ps = pmix.tile([1, 512], f32, tag="pmix")
                    nc.tensor.matmul(ps[:], negHalfCol[0:CI, :],
                                     ysq[0:CI, nj * 512:(nj + 1) * 512],
                                     start=True, stop=True)
                    nc.scalar.activation(negxx[:, nj * 512:(nj + 1) * 512],
                                         ps[:], AF.Copy)

                # ---------- stat psums (locked for the layer) ----------
                psSQu = pstat.tile([33, 512], f32, tag="psSQ", name="psSQu")
                psSu = psSQu[0:1, :]
                psQu = psSQu[32:33, :]
                psX = [pstat.tile([CH, 512], f32, tag=f"psX{h}", name=f"psX{li}_{h}") for h in range(NH)]

                yPre = None
                if not last_layer:
                    yPre = big.tile([CH, N], f32, tag="big", name=f"yPre{li}")

                vtab = dram.tile([N, CO], gdt, tag=f"vtab{li}")
                u_all = allp.tile([128, NT, CO], f32, tag="u_all")
                ubf_all = allp.tile([128, NT, CO], bf16, tag="ubf_all")

                st = {}   # per-tile tiles: S, maxv, idxu, dst

                def scores_phase(t):
                    tsl = slice(t * 128, (t + 1) * 128)
                    S = big.tile([128, N], f32, tag="big", name=f"S{li}_{t}")
                    if aug_scores:
                        aug = small.tile([CIa + 1, 128], f32, tag=f"aug{li}",
                                         name=f"aug{li}_{t}")
                        nc.scalar.activation(aug[0:CIa, :], yT[0:CIa, tsl], AF.Copy)
                        nc.vector.memset(aug[CIa:CIa + 1, :], 1.0)
                    for nj in range(4):
                        psSc = pscore.tile([128, 512], f32, tag="psc")
                        if aug_scores:
                            nc.tensor.matmul(psSc[:], aug[:],
                                             yT[0:CIa + 1, nj * 512:(nj + 1) * 512],
                                             start=True, stop=True)
                        else:
                            nc.tensor.matmul(psSc[:], yT[0:CI, tsl],
                                             yT[0:CI, nj * 512:(nj + 1) * 512],
                                             start=True, stop=False)
                            nc.tensor.matmul(psSc[:], onesRow[:, 0:128],
                                             negxx[:, nj * 512:(nj + 1) * 512],
                                             start=False, stop=True)
                        nc.scalar.activation(S[:, nj * 512:(nj + 1) * 512],
                                             psSc[:], AF.Copy)
                    st[t] = {"S": S}

                def topk_phase(t):
                    S = st[t]["S"]
                    maxv = small.tile([128, 24], f32, tag="maxv")
                    idxu = small.tile([128, 24], u32, tag="idxu")
                    for r in range(3):
                        rs = slice(8 * r, 8 * (r + 1))
                        nc.vector.max(maxv[:, rs], S[:])
                        nc.vector.max_index(idxu[:, rs], maxv[:, rs], S[:])
                        if r < 2:
                            nc.vector.match_replace(S[:], maxv[:, rs], S[:], -1e30)
                    st[t]["idxu"] = idxu

                def plumb_phase(t):
                    # index plumbing [128,20] u32 -> wrapped [128,160] i16.
                    # Emitted one iteration after topk(t) so the transposes
                    # never head-of-line-block the PE queue.
                    idxu = st[t]["idxu"]
                    idxf = small.tile([128, 20], f32, tag="idxf")
                    nc.vector.tensor_copy(idxf[:], idxu[:, 0:20])
                    psT1 = pmix.tile([20, 128], f32, tag="pmix")
                    nc.tensor.transpose(psT1[:], idxf[:], ident[:])
                    idxT = small.tile([20, 128], f32, tag="idxT")
                    nc.scalar.activation(idxT[:], psT1[:], AF.Copy)
                    psT2 = pmix.tile([16, 8, 20], f32, tag="pmix")
                    for pg in range(8):
                        nc.tensor.transpose(psT2[:, pg, :],
                                            idxT[:, pg * 16:(pg + 1) * 16],
                                            ident[0:20, 0:20])
                    idxs16 = small.tile([128, 160], i16, tag="idxs16")
                    nc.scalar.activation(
                        idxs16[0:16, :].rearrange("q (c pg) -> q pg c", pg=8),
                        psT2[:], AF.Copy)
                    nc.sync.dma_start(idxs16[16:32, :], idxs16[0:16, :])
                    nc.sync.dma_start(idxs16[32:64, :], idxs16[0:32, :])
                    nc.sync.dma_start(idxs16[64:128, :], idxs16[0:64, :])
                    dst = dstp.tile([128, K, CO], gdt, tag="dst", name=f"dst{li}_{t}")
                    for qi, (off, n) in enumerate(GATHER_SPLITS):
                        nc.gpsimd.dma_gather(
                            dst[:, off // 128:(off + n) // 128, :], vtab[:],
                            idxs16[:, off // 16:(off + n) // 16], n, n, CO,
                            queue_num=qi)
                    st[t]["dst"] = dst

                def back(t):
                    dst = st.pop(t)["dst"]
                    tsl = slice(t * 128, (t + 1) * 128)
                    if gdt is bf16:
                        dbf = dst
                        dsq = bfp.tile([128, K, CO], bf16, tag="dsq")
                        nc.scalar.activation(dsq[:], dst[:], AF.Square)
                        # m = max_j dst: 2x bf16 max tree 20->10->5->(2,2,1)->1
                        mt1 = bfp.tile([128, 10, CO], bf16, tag="mt1")
                        nc.vector.tensor_tensor(out=mt1[:], in0=dst[:, 0:10, :],
                                                in1=dst[:, 10:20, :], op=AOT.max)
                        mt2 = vcp.tile([128, 5, CO], bf16, tag="mt2")
                        nc.vector.tensor_tensor(out=mt2[:], in0=mt1[:, 0:5, :],
                                                in1=mt1[:, 5:10, :], op=AOT.max)
                        nc.vector.tensor_tensor(out=mt2[:, 0:2, :],
                                                in0=mt2[:, 0:2, :],
                                                in1=mt2[:, 2:4, :], op=AOT.max)
                        nc.vector.tensor_tensor(out=mt2[:, 0:1, :],
                                                in0=mt2[:, 0:1, :],
                                                in1=mt2[:, 1:2, :], op=AOT.max)
                        mloc = vcp.tile([128, CO], f32, tag="mloc")
                        nc.vector.tensor_tensor(out=mloc[:],
                                                in0=mt2[:, 0:1, :],
                                                in1=mt2[:, 4:5, :], op=AOT.max)
                    else:
                        dbf = bfp.tile([128, K, CO], bf16, tag="dbf")
                        nc.scalar.activation(dbf[:], dst[:], AF.Copy)
                        dsq = bfp.tile([128, K, CO], bf16, tag="dsq")
                        nc.scalar.activation(dsq[:], dst[:], AF.Square)
                        mloc = vcp.tile([128, CO], f32, tag="mloc")
                        nc.vector.tensor_reduce(mloc[:],
                                                dst[:].rearrange("p j c -> p c j"),
                                                mybir.AxisListType.X, AOT.max)
                    # stats matmuls (bf16), grouped by stationary operand
                    first = (t == 0)
                    last = (t == NT - 1)
                    for gi, (j0, gn) in enumerate(jgroups):
                        w = gn * CO
                        nc.tensor.matmul(psS[:, 0:w], onesCol_bf[:],
                                         dbf[:, j0:j0 + gn, :],
                                         start=(first and gi == 0),
                                         stop=(last and gi == len(jgroups) - 1),
                                         skip_group_check=True)
                    for gi, (j0, gn) in enumerate(jgroups):
                        w = gn * CO
                        nc.tensor.matmul(psQ[:, 0:w], onesCol_bf[:],
                                         dsq[:, j0:j0 + gn, :],
                                         start=(first and gi == 0),
                                         stop=(last and gi == len(jgroups) - 1),
                                         skip_group_check=True)
                    for h in range(NH):
                        for gi, (j0, gn) in enumerate(jgroups):
                            w = gn * CO
                            nc.tensor.matmul(
                                psX[h][:, 0:w],
                                ubf_all[:, t, 128 * h:128 * h + CH],
                                dbf[:, j0:j0 + gn, :],
                                start=(first and gi == 0),
                                stop=(last and gi == len(jgroups) - 1),
                                skip_group_check=True)
                    # pre-barrier y: wsum, transpose, stage into yPre / gmax
                    wsum = vcp.tile([128, CO], f32, tag="wsum")
                    nc.vector.tensor_tensor(out=wsum[:], in0=u_all[:, t, :],
                                            in1=mloc[:], op=AOT.add)
                    for h in range(NH):
                        psY = pyp.tile([128, 128], f32, tag="pyp")
                        nc.tensor.transpose(psY[0:CH, :],
                                            wsum[:, 128 * h:128 * h + CH],
                                            ident[:])
                        if not last_layer:
                            nc.scalar.activation(yPre[:, tsl], psY[0:CH, :],
                                                 AF.Copy)
                        else:
                            nc.vector.tensor_tensor(out=gmax[:, h, :],
                                                    in0=gmax[:, h, :],
                                                    in1=psY[0:CH, :], op=AOT.max)

                # tile 0 scores+topk first so the DVE starts immediately;
                # u/v table prep then overlaps topk(0) on PE/ACT.
                scores_phase(0)
                topk_phase(0)

                for t in range(NT):
                    tsl = slice(t * 128, (t + 1) * 128)
                    psU = pmix.tile([128, CO], f32, tag="pmix")
                    nc.tensor.matmul(psU[:], yT[0:CI, tsl], wcm[:], start=True, stop=False)
                    nc.tensor.matmul(psU[:], onesRow[:, 0:128], brow[:],
                                     start=False, stop=True)
                    nc.scalar.activation(u_all[:, t, :], psU[:], AF.Copy)
                    nc.scalar.activation(ubf_all[:, t, :], psU[:], AF.Copy)
                    psV = pmix.tile([128, CO], f32, tag="pmix")
                    nc.tensor.matmul(psV[:], yT[0:CI, tsl], wn[:],
                                     start=True, stop=True)
                    vst = vcp.tile([128, CO], gdt, tag="vst")
                    nc.scalar.activation(vst[:], psV[:], AF.Copy)
                    nc.sync.dma_start(vtab[tsl, :], vst[:])

                # Sum u / Sum u^2 (fp32, exact)
                u_flat = u_all[:].rearrange("p t c -> p (t c)")
                nuv = NT * CO // 512
                for s in range(nuv):
                    usqf = vcp.tile([128, 512], f32, tag="usq")
                    nc.scalar.activation(usqf[:], u_flat[:, 512 * s:512 * (s + 1)],
                                         AF.Square)
                    nc.tensor.matmul(psSu, onesColF[:],
                                     u_flat[:, 512 * s:512 * (s + 1)],
                                     start=(s == 0), stop=(s == nuv - 1),
                                     skip_group_check=True)
                    nc.tensor.matmul(psQu, onesColF[:], usqf[:],
                                     start=(s == 0), stop=(s == nuv - 1),
                                     skip_group_check=True)
                rowSu = rows.tile([1, 512], f32, tag="rowSu")
                nc.scalar.activation(rowSu[:], psSu, AF.Copy)
                rowQu = rows.tile([1, 512], f32, tag="rowQu")
                nc.scalar.activation(rowQu[:], psQu, AF.Copy)
                for row in (rowSu, rowQu):
                    wfull = 512
                    while wfull > CO:
                        half = wfull // 2
                        nc.vector.tensor_tensor(out=row[:, 0:half],
                                                in0=row[:, 0:half],
                                                in1=row[:, half:wfull], op=AOT.add)
                        wfull = half

                psSQ = pstat.tile([33, 512], f32, tag="psSQ", name="psSQm")
                psS = psSQ[0:1, :]
                psQ = psSQ[32:33, :]

                # 2-deep pipeline: per iteration the PE queue sees only
                # ready work (plumb(it-1) transposes depend on a topk that
                # finished last iteration -> no head-of-line stalls).
                for it in range(1, NT + 2):
                    if it - 1 < NT:
                        plumb_phase(it - 1)
                    if it < NT:
                        scores_phase(it)
                    if it >= 2:
                        back(it - 2)
                    if it < NT:
                        topk_phase(it)

                # ---------- copy out S/Q, then fold 512 -> CO ----------
                rowS = rows.tile([1, 512], f32, tag="rowS")
                nc.scalar.activation(rowS[:], psS, AF.Copy)
                rowQ = rows.tile([1, 512], f32, tag="rowQ")
                nc.scalar.activation(rowQ[:], psQ, AF.Copy)

                for row in (rowS, rowQ):
                    wfull = 512
                    while wfull > CO:
                        half = wfull // 2
                        nc.vector.tensor_tensor(out=row[:, 0:half],
                                                in0=row[:, 0:half],
                                                in1=row[:, half:wfull], op=AOT.add)
                        wfull = half

                # cross-term: diag of psX via ttr with mask, then -> row
                junk = small.tile([128, 512], f32, tag="junk")
                crossRow = rows.tile([1, 256], f32, tag="crossRow")
                for h in range(NH):
                    ccol = small.tile([128, 1], f32, tag="ccol")
                    nc.vector.tensor_tensor(out=junk[0:CH, :], in0=psX[h][:],
                                            in1=masks[h][:], op=AOT.mult)
                    nc.vector.tensor_reduce(ccol[0:CH, :], junk[0:CH, :],
                                            mybir.AxisListType.X, AOT.add)
                    psCr = pmix.tile([1, CH], f32, tag="pmix")
                    nc.tensor.transpose(psCr[:], ccol[0:CH, :], ident[0:CH, 0:CH])
                    nc.scalar.activation(crossRow[:, 128 * h:128 * h + CH],
                                         psCr[:], AF.Copy)

                # ---------- per-core partial sums -> allreduce ----------
                statsrow = rows.tile([1, 512], f32, tag="statsrow")
                nc.vector.tensor_scalar(out=statsrow[:, 0:CO], in0=rowSu[:, 0:CO],
                                        scalar1=float(K), scalar2=None,
                                        op0=AOT.mult)
                nc.vector.tensor_tensor(out=statsrow[:, 0:CO],
                                        in0=statsrow[:, 0:CO],
                                        in1=rowS[:, 0:CO], op=AOT.add)
                nc.vector.tensor_scalar(out=statsrow[:, CO:2 * CO],
                                        in0=rowQu[:, 0:CO], scalar1=float(K),
                                        scalar2=None, op0=AOT.mult)
                nc.vector.tensor_scalar(out=crossRow[:, 0:CO], in0=crossRow[:, 0:CO],
                                        scalar1=2.0, scalar2=None, op0=AOT.mult)
                nc.vector.tensor_tensor(out=statsrow[:, CO:2 * CO],
                                        in0=statsrow[:, CO:2 * CO],
                                        in1=crossRow[:, 0:CO], op=AOT.add)
                nc.vector.tensor_tensor(out=statsrow[:, CO:2 * CO],
                                        in0=statsrow[:, CO:2 * CO],
                                        in1=rowQ[:, 0:CO], op=AOT.add)

                ccin = dram.tile([1, 2 * CO], f32, tag=f"ccin{li}")
                ccout = dram.tile([1, 2 * CO], f32, tag=f"ccout{li}")
                nc.sync.dma_start(ccin[:], statsrow[:, 0:2 * CO])
                nc.gpsimd.collective_compute(
                    "AllReduce", AOT.add,
                    replica_groups=[list(range(NCORES))],
                    ins=[ccin.opt()], outs=[ccout.opt()])
                statsg = rows.tile([1, 512], f32, tag="statsg")
                nc.sync.dma_start(statsg[:, 0:2 * CO], ccout[:])

                # ---------- BN scale/shift ----------
                cntr = 1.0 / float(B * N * K)
                meanR = rows.tile([1, 256], f32, tag="meanR")
                nc.vector.tensor_scalar(out=meanR[:, 0:CO], in0=statsg[:, 0:CO],
                                        scalar1=cntr, scalar2=None, op0=AOT.mult)
                t1R = rows.tile([1, 256], f32, tag="t1R")
                t2R = rows.tile([1, 256], f32, tag="t2R")
                nc.vector.tensor_scalar(out=t1R[:, 0:CO], in0=statsg[:, CO:2 * CO],
                                        scalar1=cntr, scalar2=None, op0=AOT.mult)
                nc.vector.tensor_tensor(out=t2R[:, 0:CO], in0=meanR[:, 0:CO],
                                        in1=meanR[:, 0:CO], op=AOT.mult)
                nc.vector.tensor_tensor(out=t1R[:, 0:CO], in0=t1R[:, 0:CO],
                                        in1=t2R[:, 0:CO], op=AOT.subtract)
                nc.vector.tensor_scalar(out=t1R[:, 0:CO], in0=t1R[:, 0:CO],
                                        scalar1=1e-5, scalar2=None, op0=AOT.add)
                nc.scalar.activation(t2R[:, 0:CO], t1R[:, 0:CO], AF.Sqrt)
                nc.vector.reciprocal(t1R[:, 0:CO], t2R[:, 0:CO])
                scaleR = rows.tile([1, 256], f32, tag="scaleR")
                nc.vector.tensor_tensor(out=scaleR[:, 0:CO], in0=grow[:].bitcast(f32),
                                        in1=t1R[:, 0:CO], op=AOT.mult)
                shiftR = rows.tile([1, 256], f32, tag="shiftR")
                nc.vector.tensor_tensor(out=shiftR[:, 0:CO], in0=meanR[:, 0:CO],
                                        in1=scaleR[:, 0:CO], op=AOT.mult)
                nc.vector.tensor_tensor(out=shiftR[:, 0:CO], in0=berow[:],
                                        in1=shiftR[:, 0:CO], op=AOT.subtract)

                scol = allp.tile([128, 2], f32, tag="scol")
                tcol = allp.tile([128, 2], f32, tag="tcol")
                for h in range(NH):
                    psc = pmix.tile([128, 1], f32, tag="pmix")
                    nc.tensor.transpose(psc[0:CH, :],
                                        scaleR[:, 128 * h:128 * h + CH],
                                        ident[0:1, 0:1])
                    nc.scalar.activation(scol[0:CH, h:h + 1], psc[0:CH, :], AF.Copy)
                    psc2 = pmix.tile([128, 1], f32, tag="pmix")
                    nc.tensor.transpose(psc2[0:CH, :],
                                        shiftR[:, 128 * h:128 * h + CH],
                                        ident[0:1, 0:1])
                    nc.scalar.activation(tcol[0:CH, h:h + 1], psc2[0:CH, :], AF.Copy)

                # ---------- y-phase (post-barrier): single affine+relu ----------
                if not last_layer:
                    yTn = ytp.tile([128, N], f32, tag=f"yt{(li + 1) % 2}")
                    nc.scalar.activation(yTn[0:CH, :], yPre[:, :], AF.Relu,
                                         bias=tcol[0:CH, 0:1],
                                         scale=scol[0:CH, 0:1])
                    yT = yTn

            # ---------- head ----------
            psH = pmix.tile([1, 256], f32, tag="pmix")
            for h in range(2):
                gcol = small.tile([128, 1], f32, tag="ccol")
                nc.vector.tensor_reduce(gcol[:], gmax[:, h, :],
                                        mybir.AxisListType.X, AOT.max)
                nc.vector.tensor_scalar(out=gcol[:], in0=gcol[:],
                                        scalar1=scol[:, h:h + 1],
                                        scalar2=tcol[:, h:h + 1],
                                        op0=AOT.mult, op1=AOT.add)
                nc.vector.tensor_scalar_max(gcol[:], gcol[:], 0.0)
                nc.tensor.matmul(psH[:], gcol[:], woT_sb[:, h, :],
                                 start=(h == 0), stop=False,
                                 skip_group_check=True)
            nc.tensor.matmul(psH[:], onesRow[:, 0:1], boRow[:],
                             start=False, stop=True, skip_group_check=True)
            outSb = rows.tile([1, 256], f32, tag="crossRow")
            nc.scalar.activation(outSb[:], psH[:], AF.Copy)
            nc.sync.dma_start(out_ext[:], outSb[:])

    nc.compile()
    return nc


def _host_prep(x, weights):
    """Build per-core input maps. x: [B, N, 3]."""
    shared = {}
    for li, (ci, co) in enumerate(LAYERS):
        W = np.asarray(weights[f"w{li + 1}"])            # [co, 2*ci]
        wc, wnn = W[:, :ci], W[:, ci:]
        shared[f"wcm{li}"] = np.ascontiguousarray((wc - wnn).T.astype(np.float32))
        shared[f"wn{li}"] = np.ascontiguousarray(wnn.T.astype(np.float32))
        shared[f"brow{li}"] = np.asarray(weights[f"b{li + 1}"]).reshape(1, co).astype(np.float32)
        shared[f"grow{li}"] = np.asarray(weights[f"g{li + 1}"]).reshape(1, co).astype(np.float32)
        shared[f"berow{li}"] = np.asarray(weights[f"be{li + 1}"]).reshape(1, co).astype(np.float32)
        G = 512 // co
        for h in range(-(-co // 128)):
            hc = min(128, co - 128 * h)
            mk = np.zeros((hc, 512), np.float32)
            for p in range(hc):
                for j in range(G):
                    mk[p, j * co + p + 128 * h] = 1.0
            shared[f"mask{li}_{h}"] = mk
    shared["ident"] = np.eye(128, dtype=np.float32)
    shared["woT"] = np.ascontiguousarray(np.asarray(weights["wo"]).T.astype(np.float32))
    shared["boRow"] = np.asarray(weights["bo"]).reshape(1, 256).astype(np.float32)
    ins = []
    for c in range(NCORES):
        m = dict(shared)
        m["xT"] = np.ascontiguousarray(np.asarray(x[c]).T.astype(np.float32))
        ins.append(m)
    return ins


def kernel(**inputs):
    from concourse.bass_utils import run_bass_kernel_spmd
    x = np.asarray(inputs["x"])
    if "nc" not in _BUILT:
        _BUILT["nc"] = _build()
    nc = _BUILT["nc"]
    in_maps = _host_prep(x, inputs)
    res = run_bass_kernel_spmd(nc, in_maps, list(range(NCORES))).results
    out = np.stack([res[c]["out"][0] for c in range(NCORES)], axis=0)
    return out.astype(np.float32)


# revision 18
# speedup vs baseline: 1.5035x; 1.0275x over previous
"""DynamicGraphCNN (DGCNN) forward pass on 8 Trainium2 NeuronCores.

Data-parallel over batch B=8: one point cloud per core. Per layer (edge-conv):
  scores  S'[i,j] = <x_i, x_j> - ||x_j||^2/2    (rank-equivalent to -dist^2)
  top-20 neighbors per row via DVE max/max_index/match_replace
  h[i,j] = u_i + v_{n(i,j)} with u = x(Wc-Wn)^T + b, v = x Wn^T
  BN (training stats over B,N,k) from global sums:
      Sum h   = 20*Sum u + Sum_{ij} v_n
      Sum h^2 = 20*Sum u^2 + 2*Sum_i u_i.s_i + Sum_{ij} v_n^2
  computed with bf16 PE matmuls over the gathered tiles (j-packed into psum),
  cross-term via u-weighted matmuls + diagonal-mask extraction.
  Cross-core reduction: one 8-core AllReduce per layer.
  y_i = relu(scale*(u_i + max_j v_n) + shift)   (monotone: max before affine)
Final: global max over points, then linear head.

v2: negxx fused into the score matmul via an augmented lhs row (L1/L2),
4-way SWDGE queue spread for the gathers, bf16 gather tables for L2/L3
(halving gather DMA and enabling 2x bf16 max-trees), stats matmuls grouped
by stationary operand to reuse loaded weights.
"""
import sys
sys.path.insert(0, '/opt/trn_rl_repo')

import numpy as np

B, N, K = 8, 2048, 20
NT = N // 128                      # 16 point tiles of 128
LAYERS = [(3, 64), (64, 128), (128, 256)]
NCORES = 8
GATHER_SPLITS = [(0, 640), (640, 640), (1280, 640), (1920, 640)]

_BUILT = {}


def _build(dbg=False):
    import contextlib
    import concourse.bacc as bacc
    import concourse.mybir as mybir
    import concourse.tile as tile

    f32 = mybir.dt.float32
    bf16 = mybir.dt.bfloat16
    i16 = mybir.dt.int16
    u32 = mybir.dt.uint32
    AOT = mybir.AluOpType
    AF = mybir.ActivationFunctionType

    nc = bacc.Bacc("TRN2", target_bir_lowering=False, debug=False,
                   num_devices=NCORES, num_swdge_queues=4)

    # ---------------- external tensors ----------------
    xT_in = nc.dram_tensor("xT", [3, N], f32, kind="ExternalInput")
    ext = {}
    for li, (ci, co) in enumerate(LAYERS):
        ext[f"wcm{li}"] = nc.dram_tensor(f"wcm{li}", [ci, co], f32, kind="ExternalInput")
        ext[f"wn{li}"] = nc.dram_tensor(f"wn{li}", [ci, co], f32, kind="ExternalInput")
        for rn in ("brow", "grow", "berow"):
            ext[f"{rn}{li}"] = nc.dram_tensor(f"{rn}{li}", [1, co], f32, kind="ExternalInput")
        for h in range(-(-co // 128)):
            hc = min(128, co - 128 * h)
            ext[f"mask{li}_{h}"] = nc.dram_tensor(
                f"mask{li}_{h}", [hc, 512], f32, kind="ExternalInput")
    ident_in = nc.dram_tensor("ident", [128, 128], f32, kind="ExternalInput")
    woT_in = nc.dram_tensor("woT", [256, 256], f32, kind="ExternalInput")
    bo_in = nc.dram_tensor("boRow", [1, 256], f32, kind="ExternalInput")
    out_ext = nc.dram_tensor("out", [1, 256], f32, kind="ExternalOutput")

    with tile.TileContext(nc) as tc:
        ctx = contextlib.ExitStack()
        with ctx:
            big = ctx.enter_context(tc.tile_pool(name="big", bufs=3))      # S / ysq
            ytp = ctx.enter_context(tc.tile_pool(name="ytp", bufs=1))      # yT (2 tags)
            allp = ctx.enter_context(tc.tile_pool(name="allp", bufs=1))    # layer residents
            resid = ctx.enter_context(tc.tile_pool(name="resid", bufs=1))  # constants
            dstp = ctx.enter_context(tc.tile_pool(name="dstp", bufs=2))
            bfp = ctx.enter_context(tc.tile_pool(name="bfp", bufs=2))      # dsq / trees
            small = ctx.enter_context(tc.tile_pool(name="small", bufs=2))  # idx plumbing
            rows = ctx.enter_context(tc.tile_pool(name="rows", bufs=1))    # [1,*] rows
            vcp = ctx.enter_context(tc.tile_pool(name="vcp", bufs=2))      # staging
            dram = ctx.enter_context(tc.tile_pool(name="dram", bufs=1, space="DRAM"))
            pscore = ctx.enter_context(tc.tile_pool(name="pscore", bufs=2, space="PSUM"))
            pyp = ctx.enter_context(tc.tile_pool(name="pyp", bufs=1, space="PSUM"))
            pmix = ctx.enter_context(tc.tile_pool(name="pmix", bufs=2, space="PSUM"))
            pstat = ctx.enter_context(tc.tile_pool(name="pstat", bufs=1, space="PSUM"))

            # ---------- kernel-lifetime constants ----------
            ident = resid.tile([128, 128], f32, tag="ident")
            nc.sync.dma_start(ident[:], ident_in[:])
            onesRow = resid.tile([1, 128], f32, tag="onesRow")
            nc.vector.memset(onesRow[:], 1.0)
            onesColF = resid.tile([128, 1], f32, tag="onesColF")
            nc.vector.memset(onesColF[:], 1.0)
            onesCol_bf = resid.tile([128, 1], bf16, tag="onesColbf")
            nc.vector.memset(onesCol_bf[:], 1.0)
            negHalfCol = resid.tile([128, 1], f32, tag="negHalfCol")
            nc.vector.memset(negHalfCol[:], -0.5)
            woT_sb = resid.tile([128, 2, 256], f32, tag="woT")
            for h in range(2):
                nc.sync.dma_start(woT_sb[:, h, :], woT_in[128 * h:128 * (h + 1), :])
            boRow = resid.tile([1, 256], f32, tag="boRow")
            nc.sync.dma_start(boRow[:], bo_in[:])
            gmax = resid.tile([128, 2, 128], f32, tag="gmax")
            nc.vector.memset(gmax[:], -1e30)

            # yT carries the CI feature rows plus one negxx row for L1/L2 so
            # the -||x_j||^2/2 bias rides the score matmul as an extra
            # contraction row. Engine accesses need 32-aligned base
            # partitions, so L1 (CI=3) pads rows 3..31 with zeros and puts
            # negxx at row 32; L2 (CI=64) puts it at row 64. L3 (CI=128) has
            # no spare partition and keeps the separate bias matmul.
            yT = ytp.tile([128, N], f32, tag="yt0")
            nc.vector.memset(yT[0:33, :], 0.0)
            nc.sync.dma_start(yT[0:3, :], xT_in[:])

            for li, (CI, CO) in enumerate(LAYERS):
                NH = -(-CO // 128)
                CH = min(128, CO)
                G = 512 // CO
                jgroups = []
                j0 = 0
                while j0 < K:
                    jgroups.append((j0, min(G, K - j0)))
                    j0 += G
                last_layer = (li == len(LAYERS) - 1)
                # fused negxx: contraction rows 0..CIa-1 are features (+ zero
                # padding), row CIa is negxx; CIa must be 32-aligned.
                CIa = -32 * (-CI // 32)
                aug_scores = (CIa + 1 <= 128)
                gdt = bf16 if (CO * 2) % 256 == 0 else f32   # gather dtype

                # ---------- weights / rows ----------
                wcm = allp.tile([CI, CO], f32, tag="wcm")
                nc.sync.dma_start(wcm[:], ext[f"wcm{li}"][:])
                wn = allp.tile([CI, CO], f32, tag="wn")
                nc.sync.dma_start(wn[:], ext[f"wn{li}"][:])
                brow = allp.tile([1, CO], f32, tag="brow")
                nc.sync.dma_start(brow[:], ext[f"brow{li}"][:])
                grow = allp.tile([1, CO], f32, tag="grow")
                nc.sync.dma_start(grow[:], ext[f"grow{li}"][:])
                berow = allp.tile([1, CO], f32, tag="berow")
                nc.sync.dma_start(berow[:], ext[f"berow{li}"][:])
                masks = []
                for h in range(NH):
                    mk = allp.tile([CH, 512], f32, tag=f"mask{h}")
                    nc.sync.dma_start(mk[:], ext[f"mask{li}_{h}"][:])
                    masks.append(mk)

                # ---------- prep: negxx row via PE ----------
                ysq = big.tile([128, N], f32, tag="big")
                nc.scalar.activation(ysq[0:CI, :], yT[0:CI, :], AF.Square)
                if aug_scores:
                    negxx = yT[CIa:CIa + 1, :]
                else:
                    negxx3 = allp.tile([1, N], f32, tag="negxx")
                    negxx = negxx3[:]
                for nj in range(4):
                    ps = pmix.tile([1, 512], f32, tag="pmix")
                    nc.tensor.matmul(ps[:], negHalfCol[0:CI, :],
                                     ysq[0:CI, nj * 512:(nj + 1) * 512],
                                     start=True, stop=True)
                    nc.scalar.activation(negxx[:, nj * 512:(nj + 1) * 512],
                                         ps[:], AF.Copy)

                # ---------- stat psums (locked for the layer) ----------
                psSQu = pstat.tile([33, 512], f32, tag="psSQ", name="psSQu")
                psSu = psSQu[0:1, :]
                psQu = psSQu[32:33, :]
                psX = [pstat.tile([CH, 512], f32, tag=f"psX{h}", name=f"psX{li}_{h}") for h in range(NH)]

                yPre = None
                if not last_layer:
                    yPre = big.tile([CH, N], f32, tag="big", name=f"yPre{li}")

                vtab = dram.tile([N, CO], gdt, tag=f"vtab{li}")
                u_all = allp.tile([128, NT, CO], f32, tag="u_all")
                ubf_all = allp.tile([128, NT, CO], bf16, tag="ubf_all")

                st = {}   # per-tile tiles: S, maxv, idxu, dst

                def scores_phase(t):
                    tsl = slice(t * 128, (t + 1) * 128)
                    S = big.tile([128, N], f32, tag="big", name=f"S{li}_{t}")
                    if aug_scores:
                        aug = small.tile([CIa + 1, 128], f32, tag=f"aug{li}",
                                         name=f"aug{li}_{t}")
                        nc.scalar.activation(aug[0:CIa, :], yT[0:CIa, tsl], AF.Copy)
                        if t < 2:
                            # the two rotating aug buffers keep their ones row
                            # across reuses; only the feature rows are rewritten
                            nc.vector.memset(aug[CIa:CIa + 1, :], 1.0)
                    for nj in range(4):
                        psSc = pscore.tile([128, 512], f32, tag="psc")
                        if aug_scores:
                            nc.tensor.matmul(psSc[:], aug[:],
                                             yT[0:CIa + 1, nj * 512:(nj + 1) * 512],
                                             start=True, stop=True)
                        else:
                            nc.tensor.matmul(psSc[:], yT[0:CI, tsl],
                                             yT[0:CI, nj * 512:(nj + 1) * 512],
                                             start=True, stop=False)
                            nc.tensor.matmul(psSc[:], onesRow[:, 0:128],
                                             negxx[:, nj * 512:(nj + 1) * 512],
                                             start=False, stop=True)
                        nc.scalar.activation(S[:, nj * 512:(nj + 1) * 512],
                                             psSc[:], AF.Copy)
                    st[t] = {"S": S}

                def topk_phase(t):
                    S = st[t]["S"]
                    maxv = small.tile([128, 24], f32, tag="maxv")
                    idxu = small.tile([128, 24], u32, tag="idxu")
                    for r in range(3):
                        rs = slice(8 * r, 8 * (r + 1))
                        nc.vector.max(maxv[:, rs], S[:])
                        nc.vector.max_index(idxu[:, rs], maxv[:, rs], S[:])
                        if r < 2:
                            nc.vector.match_replace(S[:], maxv[:, rs], S[:], -1e30)
                        # PE keep-alive: a trivial matmul gated on each top-k
                        # round so the PE never sees a >3.4us idle window and
                        # the HAM clock-gate stays at full rate through the
                        # DVE-heavy stretch.
                        psJ = pyp.tile([8, 8], f32, tag="pyp")
                        nc.tensor.matmul(psJ[:], maxv[0:1, rs], maxv[0:1, rs],
                                         start=True, stop=True,
                                         skip_group_check=True)
                    st[t]["idxu"] = idxu

                def plumb_phase(t):
                    # index plumbing [128,20] u32 -> wrapped [128,160] i16.
                    # Emitted one iteration after topk(t) so the transposes
                    # never head-of-line-block the PE queue.
                    idxu = st[t]["idxu"]
                    idxf = small.tile([128, 20], f32, tag="idxf")
                    nc.vector.tensor_copy(idxf[:], idxu[:, 0:20])
                    psT1 = pmix.tile([20, 128], f32, tag="pmix")
                    nc.tensor.transpose(psT1[:], idxf[:], ident[:])
                    idxT = small.tile([20, 128], f32, tag="idxT")
                    nc.scalar.activation(idxT[:], psT1[:], AF.Copy)
                    psT2 = pmix.tile([16, 8, 20], f32, tag="pmix")
                    for pg in range(8):
                        nc.tensor.transpose(psT2[:, pg, :],
                                            idxT[:, pg * 16:(pg + 1) * 16],
                                            ident[0:20, 0:20])
                    idxs16 = small.tile([128, 160], i16, tag="idxs16")
                    nc.scalar.activation(
                        idxs16[0:16, :].rearrange("q (c pg) -> q pg c", pg=8),
                        psT2[:], AF.Copy)
                    nc.sync.dma_start(idxs16[16:32, :], idxs16[0:16, :])
                    nc.sync.dma_start(idxs16[32:64, :], idxs16[0:32, :])
                    nc.sync.dma_start(idxs16[64:128, :], idxs16[0:64, :])
                    dst = dstp.tile([128, K, CO], gdt, tag="dst", name=f"dst{li}_{t}")
                    for qi, (off, n) in enumerate(GATHER_SPLITS):
                        nc.gpsimd.dma_gather(
                            dst[:, off // 128:(off + n) // 128, :], vtab[:],
                            idxs16[:, off // 16:(off + n) // 16], n, n, CO,
                            queue_num=qi)
                    st[t]["dst"] = dst

                def back(t):
                    dst = st.pop(t)["dst"]
                    tsl = slice(t * 128, (t + 1) * 128)
                    if gdt is bf16:
                        dbf = dst
                        dsq = bfp.tile([128, K, CO], bf16, tag="dsq")
                        nc.scalar.activation(dsq[:], dst[:], AF.Square)
                        # m = max_j dst: 2x bf16 max tree 20->10->5->(2,2,1)->1
                        mt1 = bfp.tile([128, 10, CO], bf16, tag="mt1")
                        nc.vector.tensor_tensor(out=mt1[:], in0=dst[:, 0:10, :],
                                                in1=dst[:, 10:20, :], op=AOT.max)
                        mt2 = vcp.tile([128, 5, CO], bf16, tag="mt2")
                        nc.vector.tensor_tensor(out=mt2[:], in0=mt1[:, 0:5, :],
                                                in1=mt1[:, 5:10, :], op=AOT.max)
                        nc.vector.tensor_tensor(out=mt2[:, 0:2, :],
                                                in0=mt2[:, 0:2, :],
                                                in1=mt2[:, 2:4, :], op=AOT.max)
                        nc.vector.tensor_tensor(out=mt2[:, 0:1, :],
                                                in0=mt2[:, 0:1, :],
                                                in1=mt2[:, 1:2, :], op=AOT.max)
                        mloc = vcp.tile([128, CO], f32, tag="mloc")
                        nc.vector.tensor_tensor(out=mloc[:],
                                                in0=mt2[:, 0:1, :],
                                                in1=mt2[:, 4:5, :], op=AOT.max)
                    else:
                        dbf = bfp.tile([128, K, CO], bf16, tag="dbf")
                        nc.scalar.activation(dbf[:], dst[:], AF.Copy)
                        dsq = bfp.tile([128, K, CO], bf16, tag="dsq")
                        nc.scalar.activation(dsq[:], dst[:], AF.Square)
                        mloc = vcp.tile([128, CO], f32, tag="mloc")
                        nc.vector.tensor_reduce(mloc[:],
                                                dst[:].rearrange("p j c -> p c j"),
                                                mybir.AxisListType.X, AOT.max)
                    # stats matmuls (bf16), grouped by stationary operand
                    first = (t == 0)
                    last = (t == NT - 1)
                    for gi, (j0, gn) in enumerate(jgroups):
                        w = gn * CO
                        nc.tensor.matmul(psS[:, 0:w], onesCol_bf[:],
                                         dbf[:, j0:j0 + gn, :],
                                         start=(first and gi == 0),
                                         stop=(last and gi == len(jgroups) - 1),
                                         skip_group_check=True)
                    for gi, (j0, gn) in enumerate(jgroups):
                        w = gn * CO
                        nc.tensor.matmul(psQ[:, 0:w], onesCol_bf[:],
                                         dsq[:, j0:j0 + gn, :],
                                         start=(first and gi == 0),
                                         stop=(last and gi == len(jgroups) - 1),
                                         skip_group_check=True)
                    for h in range(NH):
                        for gi, (j0, gn) in enumerate(jgroups):
                            w = gn * CO
                            nc.tensor.matmul(
                                psX[h][:, 0:w],
                                ubf_all[:, t, 128 * h:128 * h + CH],
                                dbf[:, j0:j0 + gn, :],
                                start=(first and gi == 0),
                                stop=(last and gi == len(jgroups) - 1),
                                skip_group_check=True)
                    # pre-barrier y: wsum, transpose, stage into yPre / gmax
                    wsum = vcp.tile([128, CO], f32, tag="wsum")
                    nc.vector.tensor_tensor(out=wsum[:], in0=u_all[:, t, :],
                                            in1=mloc[:], op=AOT.add)
                    for h in range(NH):
                        psY = pyp.tile([128, 128], f32, tag="pyp")
                        nc.tensor.transpose(psY[0:CH, :],
                                            wsum[:, 128 * h:128 * h + CH],
                                            ident[:])
                        if not last_layer:
                            nc.scalar.activation(yPre[:, tsl], psY[0:CH, :],
                                                 AF.Copy)
                        else:
                            nc.vector.tensor_tensor(out=gmax[:, h, :],
                                                    in0=gmax[:, h, :],
                                                    in1=psY[0:CH, :], op=AOT.max)

                # tile 0 scores+topk first so the DVE starts immediately;
                # u/v table prep then overlaps topk(0) on PE/ACT.
                scores_phase(0)
                topk_phase(0)

                for t in range(NT):
                    tsl = slice(t * 128, (t + 1) * 128)
                    psU = pmix.tile([128, CO], f32, tag="pmix")
                    nc.tensor.matmul(psU[:], yT[0:CI, tsl], wcm[:], start=True, stop=False)
                    nc.tensor.matmul(psU[:], onesRow[:, 0:128], brow[:],
                                     start=False, stop=True)
                    nc.scalar.activation(u_all[:, t, :], psU[:], AF.Copy)
                    nc.scalar.activation(ubf_all[:, t, :], psU[:], AF.Copy)
                    psV = pmix.tile([128, CO], f32, tag="pmix")
                    nc.tensor.matmul(psV[:], yT[0:CI, tsl], wn[:],
                                     start=True, stop=True)
                    vst = vcp.tile([128, CO], gdt, tag="vst")
                    nc.scalar.activation(vst[:], psV[:], AF.Copy)
                    nc.sync.dma_start(vtab[tsl, :], vst[:])

                # Sum u / Sum u^2 (fp32, exact)
                u_flat = u_all[:].rearrange("p t c -> p (t c)")
                nuv = NT * CO // 512
                for s in range(nuv):
                    usqf = vcp.tile([128, 512], f32, tag="usq")
                    nc.scalar.activation(usqf[:], u_flat[:, 512 * s:512 * (s + 1)],
                                         AF.Square)
                    nc.tensor.matmul(psSu, onesColF[:],
                                     u_flat[:, 512 * s:512 * (s + 1)],
                                     start=(s == 0), stop=(s == nuv - 1),
                                     skip_group_check=True)
                    nc.tensor.matmul(psQu, onesColF[:], usqf[:],
                                     start=(s == 0), stop=(s == nuv - 1),
                                     skip_group_check=True)
                rowSu = rows.tile([1, 512], f32, tag="rowSu")
                nc.scalar.activation(rowSu[:], psSu, AF.Copy)
                rowQu = rows.tile([1, 512], f32, tag="rowQu")
                nc.scalar.activation(rowQu[:], psQu, AF.Copy)
                for row in (rowSu, rowQu):
                    wfull = 512
                    while wfull > CO:
                        half = wfull // 2
                        nc.vector.tensor_tensor(out=row[:, 0:half],
                                                in0=row[:, 0:half],
                                                in1=row[:, half:wfull], op=AOT.add)
                        wfull = half

                psSQ = pstat.tile([33, 512], f32, tag="psSQ", name="psSQm")
                psS = psSQ[0:1, :]
                psQ = psSQ[32:33, :]

                # 2-deep pipeline: per iteration the PE queue sees only
                # ready work (plumb(it-1) transposes depend on a topk that
                # finished last iteration -> no head-of-line stalls).
                for it in range(1, NT + 2):
                    if it - 1 < NT:
                        plumb_phase(it - 1)
                    if it < NT:
                        scores_phase(it)
                    if it >= 2:
                        back(it - 2)
                    if it < NT:
                        topk_phase(it)

                # ---------- copy out S/Q, then fold 512 -> CO ----------
                rowS = rows.tile([1, 512], f32, tag="rowS")
                nc.scalar.activation(rowS[:], psS, AF.Copy)
                rowQ = rows.tile([1, 512], f32, tag="rowQ")
                nc.scalar.activation(rowQ[:], psQ, AF.Copy)

                for row in (rowS, rowQ):
                    wfull = 512
                    while wfull > CO:
                        half = wfull // 2
                        nc.vector.tensor_tensor(out=row[:, 0:half],
                                                in0=row[:, 0:half],
                                                in1=row[:, half:wfull], op=AOT.add)
                        wfull = half

                # cross-term: diag of psX via ttr with mask, then -> row
                junk = small.tile([128, 512], f32, tag="junk")
                crossRow = rows.tile([1, 256], f32, tag="crossRow")
                for h in range(NH):
                    ccol = small.tile([128, 1], f32, tag="ccol")
                    nc.vector.tensor_tensor(out=junk[0:CH, :], in0=psX[h][:],
                                            in1=masks[h][:], op=AOT.mult)
                    nc.vector.tensor_reduce(ccol[0:CH, :], junk[0:CH, :],
                                            mybir.AxisListType.X, AOT.add)
                    psCr = pmix.tile([1, CH], f32, tag="pmix")
                    nc.tensor.transpose(psCr[:], ccol[0:CH, :], ident[0:CH, 0:CH])
                    nc.scalar.activation(crossRow[:, 128 * h:128 * h + CH],
                                         psCr[:], AF.Copy)

                # ---------- per-core partial sums -> allreduce ----------
                statsrow = rows.tile([1, 512], f32, tag="statsrow")
                nc.vector.tensor_scalar(out=statsrow[:, 0:CO], in0=rowSu[:, 0:CO],
                                        scalar1=float(K), scalar2=None,
                                        op0=AOT.mult)
                nc.vector.tensor_tensor(out=statsrow[:, 0:CO],
                                        in0=statsrow[:, 0:CO],
                                        in1=rowS[:, 0:CO], op=AOT.add)
                nc.vector.tensor_scalar(out=statsrow[:, CO:2 * CO],
                                        in0=rowQu[:, 0:CO], scalar1=float(K),
                                        scalar2=None, op0=AOT.mult)
                nc.vector.tensor_scalar(out=crossRow[:, 0:CO], in0=crossRow[:, 0:CO],
                                        scalar1=2.0, scalar2=None, op0=AOT.mult)
                nc.vector.tensor_tensor(out=statsrow[:, CO:2 * CO],
                                        in0=statsrow[:, CO:2 * CO],
                                        in1=crossRow[:, 0:CO], op=AOT.add)
                nc.vector.tensor_tensor(out=statsrow[:, CO:2 * CO],
                                        in0=statsrow[:, CO:2 * CO],
                                        in1=rowQ[:, 0:CO], op=AOT.add)

                ccin = dram.tile([1, 2 * CO], f32, tag=f"ccin{li}")
                ccout = dram.tile([1, 2 * CO], f32, tag=f"ccout{li}")
                nc.sync.dma_start(ccin[:], statsrow[:, 0:2 * CO])
                nc.gpsimd.collective_compute(
                    "AllReduce", AOT.add,
                    replica_groups=[list(range(NCORES))],
                    ins=[ccin.opt()], outs=[ccout.opt()])
                statsg = rows.tile([1, 512], f32, tag="statsg")
                nc.sync.dma_start(statsg[:, 0:2 * CO], ccout[:])

                # ---------- BN scale/shift ----------
                cntr = 1.0 / float(B * N * K)
                meanR = rows.tile([1, 256], f32, tag="meanR")
                nc.vector.tensor_scalar(out=meanR[:, 0:CO], in0=statsg[:, 0:CO],
                                        scalar1=cntr, scalar2=None, op0=AOT.mult)
                t1R = rows.tile([1, 256], f32, tag="t1R")
                t2R = rows.tile([1, 256], f32, tag="t2R")
                nc.vector.tensor_scalar(out=t1R[:, 0:CO], in0=statsg[:, CO:2 * CO],
                                        scalar1=cntr, scalar2=None, op0=AOT.mult)
                nc.vector.tensor_tensor(out=t2R[:, 0:CO], in0=meanR[:, 0:CO],
                                        in1=meanR[:, 0:CO], op=AOT.mult)
                nc.vector.tensor_tensor(out=t1R[:, 0:CO], in0=t1R[:, 0:CO],
                                        in1=t2R[:, 0:CO], op=AOT.subtract)
                nc.vector.tensor_scalar(out=t1R[:, 0:CO], in0=t1R[:, 0:CO],
                                        scalar1=1e-5, scalar2=None, op0=AOT.add)
                nc.scalar.activation(t2R[:, 0:CO], t1R[:, 0:CO], AF.Sqrt)
                nc.vector.reciprocal(t1R[:, 0:CO], t2R[:, 0:CO])
                scaleR = rows.tile([1, 256], f32, tag="scaleR")
                nc.vector.tensor_tensor(out=scaleR[:, 0:CO], in0=grow[:].bitcast(f32),
                                        in1=t1R[:, 0:CO], op=AOT.mult)
                shiftR = rows.tile([1, 256], f32, tag="shiftR")
                nc.vector.tensor_tensor(out=shiftR[:, 0:CO], in0=meanR[:, 0:CO],
                                        in1=scaleR[:, 0:CO], op=AOT.mult)
                nc.vector.tensor_tensor(out=shiftR[:, 0:CO], in0=berow[:],
                                        in1=shiftR[:, 0:CO], op=AOT.subtract)

                scol = allp.tile([128, 2], f32, tag="scol")
                tcol = allp.tile([128, 2], f32, tag="tcol")
                for h in range(NH):
                    psc = pmix.tile([128, 1], f32, tag="pmix")
                    nc.tensor.transpose(psc[0:CH, :],
                                        scaleR[:, 128 * h:128 * h + CH],
                                        ident[0:1, 0:1])
                    nc.scalar.activation(scol[0:CH, h:h + 1], psc[0:CH, :], AF.Copy)
                    psc2 = pmix.tile([128, 1], f32, tag="pmix")
                    nc.tensor.transpose(psc2[0:CH, :],
                                        shiftR[:, 128 * h:128 * h + CH],
                                        ident[0:1, 0:1])
                    nc.scalar.activation(tcol[0:CH, h:h + 1], psc2[0:CH, :], AF.Copy)

                # ---------- y-phase (post-barrier): single affine+relu ----------
                if not last_layer:
                    yTn = ytp.tile([128, N], f32, tag=f"yt{(li + 1) % 2}")
                    nc.scalar.activation(yTn[0:CH, :], yPre[:, :], AF.Relu,
                                         bias=tcol[0:CH, 0:1],
                                         scale=scol[0:CH, 0:1])
                    yT = yTn

            # ---------- head ----------
            psH = pmix.tile([1, 256], f32, tag="pmix")
            for h in range(2):
                gcol = small.tile([128, 1], f32, tag="ccol")
                nc.vector.tensor_reduce(gcol[:], gmax[:, h, :],
                                        mybir.AxisListType.X, AOT.max)
                nc.vector.tensor_scalar(out=gcol[:], in0=gcol[:],
                                        scalar1=scol[:, h:h + 1],
                                        scalar2=tcol[:, h:h + 1],
                                        op0=AOT.mult, op1=AOT.add)
                nc.vector.tensor_scalar_max(gcol[:], gcol[:], 0.0)
                nc.tensor.matmul(psH[:], gcol[:], woT_sb[:, h, :],
                                 start=(h == 0), stop=False,
                                 skip_group_check=True)
            nc.tensor.matmul(psH[:], onesRow[:, 0:1], boRow[:],
                             start=False, stop=True, skip_group_check=True)
            outSb = rows.tile([1, 256], f32, tag="crossRow")
            nc.scalar.activation(outSb[:], psH[:], AF.Copy)
            nc.sync.dma_start(out_ext[:], outSb[:])

    nc.compile()
    return nc


def _host_prep(x, weights):
    """Build per-core input maps. x: [B, N, 3]."""
    shared = {}
    for li, (ci, co) in enumerate(LAYERS):
        W = np.asarray(weights[f"w{li + 1}"])            # [co, 2*ci]
        wc, wnn = W[:, :ci], W[:, ci:]
        shared[f"wcm{li}"] = np.ascontiguousarray((wc - wnn).T.astype(np.float32))
        shared[f"wn{li}"] = np.ascontiguousarray(wnn.T.astype(np.float32))
        shared[f"brow{li}"] = np.asarray(weights[f"b{li + 1}"]).reshape(1, co).astype(np.float32)
        shared[f"grow{li}"] = np.asarray(weights[f"g{li + 1}"]).reshape(1, co).astype(np.float32)
        shared[f"berow{li}"] = np.asarray(weights[f"be{li + 1}"]).reshape(1, co).astype(np.float32)
        G = 512 // co
        for h in range(-(-co // 128)):
            hc = min(128, co - 128 * h)
            mk = np.zeros((hc, 512), np.float32)
            for p in range(hc):
                for j in range(G):
                    mk[p, j * co + p + 128 * h] = 1.0
            shared[f"mask{li}_{h}"] = mk
    shared["ident"] = np.eye(128, dtype=np.float32)
    shared["woT"] = np.ascontiguousarray(np.asarray(weights["wo"]).T.astype(np.float32))
    shared["boRow"] = np.asarray(weights["bo"]).reshape(1, 256).astype(np.float32)
    ins = []
    for c in range(NCORES):
        m = dict(shared)
        m["xT"] = np.ascontiguousarray(np.asarray(x[c]).T.astype(np.float32))
        ins.append(m)
    return ins


def kernel(**inputs):
    from concourse.bass_utils import run_bass_kernel_spmd
    x = np.asarray(inputs["x"])
    if "nc" not in _BUILT:
        _BUILT["nc"] = _build()
    nc = _BUILT["nc"]
    in_maps = _host_prep(x, inputs)
    res = run_bass_kernel_spmd(nc, in_maps, list(range(NCORES))).results
    out = np.stack([res[c]["out"][0] for c in range(NCORES)], axis=0)
    return out.astype(np.float32)
